# revision 1
# baseline (speedup 1.0000x reference)
"""Trainium2 Bass kernel for nn_CausalSelfAttention_8237747274097.

Reference math (single-head attention over full n_embd=1024, scale 1/8):
    qkv = x @ W_attn + b_attn ; q,k,v = split(qkv)
    att = softmax(causal(q @ k.T / 8)) ; y = att @ v ; out = y @ W_proj + b_proj

Sharding (8 cores): core c = (batch b = c//2, parity p = c%2). Each core owns 8
of the 16 query row-tiles (128 rows each) of its batch, interleaved/paired so
causal work is balanced, and computes full K/V for the batch. Outputs are
disjoint row slices -> host gather is a pure scatter + bias add.

Math simplifications (all exact):
  - k bias drops out of softmax; v bias folds into b_eff = b_proj + b_v@W_proj.
  - 1/8 q scale folded into W_q/b_q.
  - W_proj folded into the value weights (associativity: (P@V)@Wp =
    P@(X@(Wv@Wp))), so the device runs NO output-projection phase; the
    flipped PV matmul (P stationary, VP moving) emits the final pre-bias
    output row-major, and the 1/den softmax scale is a per-partition
    multiply fused into the PSUM->SBUF output copy.
Softmax is computed without max-subtraction (scores are O(3); exp is safe) so
the denominator comes free from a ones-row matmul.

Precision: all matmul operands fp16 (full PE rate, FWL-eligible weight loads,
half the HBM traffic of fp32); accumulation is always fp32 in PSUM.

Phase order is chosen for DMA/compute overlap from a cold start:
  Q projection first (needs only 1.3 MB before the first matmul), then
  VP = X @ (Wv Wp), then K^T, then attention (scores -> den -> flipped PV).
"""

import numpy as np
import ml_dtypes

import concourse.bass as bass
import concourse.tile as tile
import concourse.mybir as mybir
from concourse import bacc
from concourse.bass import ts, ds
from concourse.bass_utils import run_bass_kernel_spmd

F32 = mybir.dt.float32
F16 = mybir.dt.float16

T, D = 2048, 1024
NT = T // 128          # 16 query/key tiles
DC = D // 128          # 8 contraction chunks
# own query tiles per core parity (descending pairing balances causal work)
# pair P of parity0: (CP[P]-1, CP[P]-4); parity1: (CP[P]-2, CP[P]-3)
OWN = [[15, 12, 11, 8, 7, 4, 3, 0],
       [14, 13, 10, 9, 6, 5, 2, 1]]
CP = [16, 12, 8, 4]    # j-blocks per slot-pair (uniform across cores)

_NC_CACHE = {}


def _build(repeat=1, phases=3):
    key = (repeat, phases)
    if key in _NC_CACHE:
        return _NC_CACHE[key]
    nc = bacc.Bacc("TRN2", target_bir_lowering=False, debug=False,
                   enable_asserts=False, num_devices=8)
    xT = nc.dram_tensor("xT", [D, T], F16, kind="ExternalInput").ap()
    xqT = nc.dram_tensor("xqT", [D, 1024], F16, kind="ExternalInput").ap()
    # wq/wk are host-pre-rearranged to the SBUF stationary layout:
    # wq[m][p][c*128+f] = W[c*128+p][m*128+f] so each m-tile is one
    # contiguous 256 KB DMA instead of a 1024-line scatter.
    # wq carries Wqk = (Wq/8) @ Wk^T (host-folded): scores then contract
    # G^T = Wqk^T x_q^T directly against the raw x tiles, so there is no
    # on-device K projection at all.
    wq = nc.dram_tensor("wq", [DC, 128, D], F16, kind="ExternalInput").ap()
    # wv carries Wv @ W_proj (host-folded): PV's output is then the final
    # (pre-bias, pre-1/den) result and no on-device projection is needed.
    wv = nc.dram_tensor("wv", [D, D], F16, kind="ExternalInput").ap()
    bq = nc.dram_tensor("bq", [D], F16, kind="ExternalInput").ap()
    # per pair: masks256 [2,128,256] (tj=cp-4,cp-3), masks128 [2,128,128]
    m256 = nc.dram_tensor("m256", [4, 2, 128, 256], F16, kind="ExternalInput").ap()
    m128 = nc.dram_tensor("m128", [4, 2, 128, 128], F16, kind="ExternalInput").ap()
    out = nc.dram_tensor("out", [1024, D], F16, kind="ExternalOutput").ap()
    den_dram = nc.dram_tensor("den_scratch", [1024], F32).ap()
    t_dram = nc.dram_tensor("t_scratch", [2048], F32).ap()

    with tile.TileContext(nc, pool_alloc_mode="queue") as tc:
        def body(_i=None):
            _emit(nc, tc, xT, xqT, wq, wv, bq, m256, m128, out,
                  den_dram, t_dram, phases)
        if repeat == 1:
            body()
        else:
            with tc.For_i(0, repeat, 1):
                body()
    nc.compile()
    _NC_CACHE[key] = nc
    return nc


def _emit(nc, tc, xT, xqT, wq, wv, bq, m256, m128, out, den_dram,
          t_dram, phases=3):
    with tc.tile_pool(name="pk", bufs=1) as pk_pool, \
         tc.tile_pool(name="pv", bufs=1) as pv_pool, \
         tc.tile_pool(name="pq", bufs=1) as pq_pool, \
         tc.tile_pool(name="wvp", bufs=1) as wv_pool, \
         tc.tile_pool(name="mskp", bufs=1) as msk_pool, \
         tc.tile_pool(name="small", bufs=1) as small:

        # ---- Phase Q: Q^T (own rows; needs only xqT + wq) ----
        # ic-outer so the first matmul needs only wqm[0] + xq[*][0] (~1.3 MB)
        bq_sb = small.tile([128, 8], F16, tag="bq", name="bq_sb")
        qT_sb = [[pq_pool.tile([128, 256], F16, tag=f"q{m}_{p}", name=f"qT_sb{m}_{p}")
                  for p in range(4)] for m in range(DC)]
        # one PSUM pool shared by phases Q/V/K (same tile shape) — avoids
        # pool-boundary syncs between the projection phases
        psA_cm = tc.tile_pool(name="psA", bufs=6, space="PSUM")
        psA = psA_cm.__enter__()
        with tc.tile_pool(name="xq", bufs=1) as xq_pool, \
             tc.tile_pool(name="wqm", bufs=1) as wq_pool:
            xq = [[xq_pool.tile([128, 512], F16, tag=f"xq{d}_{j}", name=f"xq{d}_{j}")
                   for j in range(2)] for d in range(DC)]
            wqm = [wq_pool.tile([128, 1024], F16, tag=f"wqm{m}", name=f"wqm{m}")
                   for m in range(DC)]

            def load_wqm(m):
                nc.sync.dma_start(wqm[m][:], wq[m, :, :])

            # DMA emission order = consumption order of the cold-start
            # m-group sweep below (the queue is serial).
            def wqm_half(m, h):
                nc.sync.dma_start(wqm[m][:, ds(512 * h, 512)],
                                  wq[m, :, ds(512 * h, 512)])

            wqm_half(0, 0)
            nc.sync.dma_start(xq[0][0][:], xqT[ts(0, 128), ts(0, 512)])
            for m in (1, 2, 3):
                wqm_half(m, 0)
            nc.sync.dma_start(bq_sb[:], bq.rearrange("(m p) -> p m", p=128))
            nc.sync.dma_start(xq[1][0][:], xqT[ts(1, 128), ts(0, 512)])
            wqm_half(0, 1)
            nc.sync.dma_start(xq[2][0][:], xqT[ts(2, 128), ts(0, 512)])
            wqm_half(1, 1)
            nc.sync.dma_start(xq[3][0][:], xqT[ts(3, 128), ts(0, 512)])
            for m in (2, 3):
                wqm_half(m, 1)
            for d in range(4, DC):
                nc.sync.dma_start(xq[d][0][:], xqT[ts(d, 128), ts(0, 512)])
            for m in range(4, DC):
                load_wqm(m)
            for d in range(DC):
                nc.sync.dma_start(xq[d][1][:], xqT[ts(d, 128), ts(1, 512)])

            if True:
                # ic=0 runs while xq/wqm stream in: m-groups of 4 with d
                # outer, so each arriving 128 KB xq chunk funds 4 matmuls —
                # compute density matches the DMA arrival rate from cold.
                for mg in (0, 4):
                    pss = [psA.tile([128, 512], F32, tag="A", name="psQ_t")
                           for _ in range(4)]
                    for d in range(DC):
                        for mi in range(4):
                            nc.tensor.matmul(pss[mi][:],
                                             wqm[mg + mi][:, ts(d, 128)],
                                             xq[d][0][:],
                                             start=(d == 0), stop=(d == DC - 1))
                    for mi in range(4):
                        for p2 in range(2):
                            nc.scalar.activation(qT_sb[mg + mi][p2][:],
                                                 pss[mi][:, ts(p2, 256)],
                                                 mybir.ActivationFunctionType.Identity)
                for m in range(DC):
                    ps = psA.tile([128, 512], F32, tag="A", name="psQ_t")
                    for d in range(DC):
                        nc.tensor.matmul(ps[:],
                                         wqm[m][:, ts(d, 128)],
                                         xq[d][1][:],
                                         start=(d == 0), stop=(d == DC - 1))
                    for p2 in range(2):
                        nc.scalar.activation(qT_sb[m][2 + p2][:],
                                             ps[:, ts(p2, 256)],
                                             mybir.ActivationFunctionType.Identity)

        # ---- xT arrives while Q computes; wv before xt so V can start ----
        # xt stays live through phase B: its tiles are the score stationaries
        xt_cm = tc.tile_pool(name="xt", bufs=1)
        xt_pool = xt_cm.__enter__()
        if True:
            wv_sb = [[wv_pool.tile([128, 512], F16, tag=f"wv{fc}_{d}",
                                   name=f"wv_sb{fc}_{d}") for d in range(DC)]
                     for fc in range(2)]
            for d in range(DC):
                nc.sync.dma_start(wv_sb[0][d][:], wv[ts(d, 128), ts(0, 512)])
            xt = [[xt_pool.tile([128, 512], F16, tag=f"xt{d}_{j}", name=f"xt{d}_{j}")
                   for j in range(4)] for d in range(DC)]
            for j in range(4):
                for d in range(DC):
                    nc.sync.dma_start(xt[d][j][:], xT[ts(d, 128), ts(j, 512)])
            for d in range(DC):
                nc.sync.dma_start(wv_sb[1][d][:], wv[ts(d, 128), ts(1, 512)])
            msk256 = [[msk_pool.tile([128, 256], F16, tag=f"m256_{P}_{i}",
                                     name=f"m256_{P}_{i}") for i in range(2)]
                      for P in range(4)]
            msk128 = [[msk_pool.tile([128, 128], F16, tag=f"m128_{P}_{i}",
                                     name=f"m128_{P}_{i}") for i in range(2)]
                      for P in range(4)]
            for P in range(4):
                for i in range(2):
                    nc.sync.dma_start(msk256[P][i][:], m256[P, i, :, :])
                    nc.sync.dma_start(msk128[P][i][:], m128[P, i, :, :])

            # ---- Phase V: V = X @ Wv/8 (full batch) ----
            v_sb = [pv_pool.tile([128, D], F16, tag=f"v{t}", name=f"v_sb{t}")
                    for t in range(NT)]
            if True:
                for tt in range(NT):
                    ps = psA.tile([128, 512], F32, tag="A", name="psV_t")
                    for d in range(DC):
                        nc.tensor.matmul(ps[:],
                                         xt[d][tt // 4][:, ts(tt % 4, 128)],
                                         wv_sb[0][d][:],
                                         start=(d == 0), stop=(d == DC - 1))
                    nc.vector.tensor_copy(v_sb[tt][:, ts(0, 512)], ps[:])
                # t[j] = x_j . (bq Wk^T/8) — the per-token additive score
                # bias (q-bias cross term under the Wqk folding); computed
                # mid-VP so its DRAM transpose round trip is fully covered
                t_sb = small.tile([128, 16], F32, tag="tsb", name="t_sb")
                t_row = small.tile([1, 2048], F32, tag="trow", name="t_row")
                for jc in range(4):
                    pst = psA.tile([1, 512], F32, tag="T", name="psT_t", bufs=1)
                    for d in range(DC):
                        nc.tensor.matmul(pst[:], bq_sb[:, d:d + 1],
                                         xt[d][jc][:],
                                         start=(d == 0), stop=(d == DC - 1))
                    nc.vector.tensor_copy(t_row[:, ds(512 * jc, 512)], pst[:])
                nc.sync.dma_start(t_dram[:], t_row[0:1, :])
                nc.sync.dma_start(t_sb[:],
                                  t_dram.rearrange("(t p) -> p t", p=128))
                for tt in range(NT):
                    ps = psA.tile([128, 512], F32, tag="A", name="psV_t")
                    for d in range(DC):
                        nc.tensor.matmul(ps[:],
                                         xt[d][tt // 4][:, ts(tt % 4, 128)],
                                         wv_sb[1][d][:],
                                         start=(d == 0), stop=(d == DC - 1))
                    nc.vector.tensor_copy(v_sb[tt][:, ts(1, 512)], ps[:])

        psA_cm.__exit__(None, None, None)

        if phases <= 1:
            with tc.tile_pool(name="dump", bufs=1) as dump:
                tk = dump.tile([128, 512], F16, tag="tk", name="tk")
                nc.vector.tensor_copy(tk[:], v_sb[1][:, 0:512])
                nc.sync.dma_start(out[0:128, 0:512], tk[:])
                tq = dump.tile([128, 512], F16, tag="tq", name="tq")
                nc.vector.tensor_copy(tq[:, 0:256], qT_sb[0][0][:])
                nc.sync.dma_start(out[0:128, 512:1024], tq[:])
                tv = dump.tile([128, 512], F16, tag="tv", name="tv")
                nc.vector.tensor_copy(tv[:], v_sb[0][:, 0:512])
                nc.sync.dma_start(out[128:256, 0:512], tv[:])
            return

        # ---- Phase B: attention + projection ----
        with tc.tile_pool(name="transB", bufs=3) as trans, \
             tc.tile_pool(name="poF", bufs=1, space="PSUM") as po_pool, \
             tc.tile_pool(name="psS", bufs=2, space="PSUM") as psS_pool, \
             tc.tile_pool(name="pden", bufs=1, space="PSUM") as pden_pool:

            ones_bf = small.tile([128, 1], F16, tag="ones", name="ones_bf")
            nc.vector.memset(ones_bf[:], 1.0)


            def width(P, tj):
                return 256 if tj < CP[P] - 2 else 128

            def emit_scores(P, proj_units=()):
                """Pure score-matmul burst (exp/mask on ACT/DVE run behind);
                proj_units for pair P-1 are interleaved at spread points."""
                cp = CP[P]
                proj_at = {}
                for g, unit in enumerate(proj_units):
                    proj_at.setdefault(max(0, cp - 1 - 2 * g), []).append(unit)
                pts = []
                for tj in range(cp):
                    w = width(P, tj)
                    psS = psS_pool.tile([128, 256], F32, tag="s", name="psS_t")
                    for d in range(DC):
                        nc.tensor.matmul(psS[:, 0:w],
                                         xt[d][tj // 4][:, ts(tj % 4, 128)],
                                         qT_sb[d][P][:, 0:w],
                                         start=(d == 0), stop=(d == DC - 1))
                    pt = trans.tile([128, 256], F16, tag=f"pt{tj}", name="pt_t",
                                    bufs=2)
                    nc.scalar.activation(pt[:, 0:w], psS[:, 0:w],
                                         mybir.ActivationFunctionType.Exp,
                                         bias=t_sb[:, tj:tj + 1])
                    mi = tj - (cp - 4)
                    if mi >= 0:
                        if w == 256:
                            nc.vector.tensor_mul(pt[:], pt[:], msk256[P][mi][:])
                        else:
                            nc.vector.tensor_mul(pt[:, 0:128], pt[:, 0:128],
                                                 msk128[P][mi - 2][:])
                    pts.append((pt, w))
                    for unit in proj_at.get(tj, ()):
                        unit()
                return pts

            def emit_den_pv(P, pts):
                cp = CP[P]
                if phases <= 2:
                    return
                # PV flipped: stationary = P-tile slot columns, moving = VP
                # rows (Wp is host-folded into VP), so the accumulator lands
                # row-major [q, dims] — it IS the final pre-bias output; the
                # per-row 1/den scale applies per-partition and no on-device
                # projection or transpose is needed.
                for slot in range(2):
                    # slot1 (the low tile of the pair) never needs the last
                    # two j-blocks; masks already zero its dead region
                    ntj = cp if slot == 0 else cp - 2
                    po = po_pool.tile([128, 1024], F32, tag=f"poF{slot}",
                                      name=f"poF{slot}_t")
                    pden = pden_pool.tile([128, 1], F32, tag=f"ds{slot}",
                                          name=f"pden{slot}_t")
                    rec = trans.tile([128, 1], F32, tag="recS", name="recS_t",
                                     bufs=4)
                    for half in range(2):
                        for tj in range(ntj):
                            pt, w = pts[tj]
                            nc.tensor.matmul(po[:, ds(512 * half, 512)],
                                             pt[:, ds(128 * slot, 128)],
                                             v_sb[tj][:, ds(512 * half, 512)],
                                             start=(tj == 0),
                                             stop=(tj == ntj - 1))
                            if half == 0:
                                # denominator rides along: same stationary,
                                # N=1, lands per-partition [q, 1]
                                nc.tensor.matmul(pden[:],
                                                 pt[:, ds(128 * slot, 128)],
                                                 ones_bf[:],
                                                 start=(tj == 0),
                                                 stop=(tj == ntj - 1))
                        if half == 0:
                            nc.vector.reciprocal(rec[:], pden[:])
                        # store this half immediately (DVE/ACT alternate):
                        # half 0's scale+DMA hides under half 1's matmuls,
                        # so only half 1's short chain remains at the tail
                        ob = trans.tile([128, 512], F16, tag="obF",
                                        name="obF_t", bufs=3)
                        rc = rec[:, 0:1]
                        if half == 0:
                            nc.vector.tensor_scalar_mul(ob[:], po[:, 0:512],
                                                        rc)
                        else:
                            nc.scalar.activation(
                                ob[:], po[:, 512:1024],
                                mybir.ActivationFunctionType.Identity,
                                scale=rc)
                        nc.sync.dma_start(
                            out[ds(128 * (2 * P + slot), 128),
                                ds(512 * half, 512)], ob[:])

            # biggest pair last: its long PV fully covers the den-reciprocal
            # round trip, so the final output scale never waits
            for P in (3, 2, 1, 0):
                pts = emit_scores(P)
                emit_den_pv(P, pts)
        xt_cm.__exit__(None, None, None)


def _host_masks(par):
    """Uniform-template masks.

    m256[P, i] multiplies the P tile at tj = cp-4+i (full 256-wide blocks);
    m128[P, i] multiplies the slot0 half at tj = cp-2+i (128-wide blocks).
    parity0 pair P owns (hi, lo) = (cp-1, cp-4); parity1 owns (cp-2, cp-3).
    """
    m256 = np.zeros((4, 2, 128, 256), np.float32)
    m128 = np.zeros((4, 2, 128, 128), np.float32)
    j = np.arange(128)[:, None]
    i = np.arange(128)[None, :]
    tri = (j <= i).astype(np.float32)   # diagonal tile mask
    ones = np.ones((128, 128), np.float32)
    zeros = np.zeros((128, 128), np.float32)
    for P in range(4):
        cp = CP[P]
        if par == 0:
            # hi = cp-1 at slot0, lo = cp-4 at slot1
            m256[P, 0, :, 0:128] = ones   # tj=cp-4 vs hi: below diag
            m256[P, 0, :, 128:256] = tri  # tj=cp-4 == lo: diagonal
            m256[P, 1, :, 0:128] = ones   # tj=cp-3 vs hi: below diag
            m256[P, 1, :, 128:256] = zeros  # tj=cp-3 > lo: dead
            m128[P, 0] = ones             # tj=cp-2 < hi
            m128[P, 1] = tri              # tj=cp-1 == hi: diagonal
        else:
            # hi = cp-2 at slot0, lo = cp-3 at slot1
            m256[P, 0, :, 0:128] = ones   # tj=cp-4 < hi
            m256[P, 0, :, 128:256] = ones  # tj=cp-4 < lo
            m256[P, 1, :, 0:128] = ones   # tj=cp-3 < hi
            m256[P, 1, :, 128:256] = tri  # tj=cp-3 == lo: diagonal
            m128[P, 0] = tri              # tj=cp-2 == hi: diagonal
            m128[P, 1] = zeros            # tj=cp-1 > hi: dead
    return m256.astype(np.float16), m128.astype(np.float16)


def kernel(x, W_attn, b_attn, W_proj, b_proj, _repeat=1, _results_only=False,
           _phases=3):
    x = np.asarray(x, np.float32)
    W_attn = np.asarray(W_attn, np.float32)
    b_attn = np.asarray(b_attn, np.float32)
    W_proj = np.asarray(W_proj, np.float32)
    b_proj = np.asarray(b_proj, np.float32)
    B = x.shape[0]

    nc = _build(_repeat, _phases)

    b_eff = (b_proj.astype(np.float64)
             + b_attn[2 * D:].astype(np.float64) @ W_proj.astype(np.float64)
             ).astype(np.float32)
    # exact pow-2 rescales: q,bq /8 (softmax scale); v /8 and wp *8 (fp16 range)
    def stat_layout(w):
        # [D, D] -> [m, p, c*128+f] with w[c*128+p][m*128+f]
        return np.ascontiguousarray(
            w.reshape(DC, 128, DC, 128).transpose(2, 1, 0, 3).reshape(DC, 128, D))

    # fold Wk into the query weights: S = x_q (Wq/8) Wk^T x^T + t[j], so
    # scores contract against raw x tiles and there is no K projection
    wqk = (W_attn[:, :D].astype(np.float64) * 0.125
           ) @ W_attn[:, D:2 * D].astype(np.float64).T
    wq = stat_layout(wqk.astype(np.float16))
    # fold the output projection into the value weights (associativity:
    # (P@V)@Wp = P@(X@(Wv@Wp))) — the device then needs no projection phase
    wv = (W_attn[:, 2 * D:].astype(np.float64)
          @ W_proj.astype(np.float64)).astype(np.float16)
    bqv = ((b_attn[:D].astype(np.float64) * 0.125)
           @ W_attn[:, D:2 * D].astype(np.float64).T).astype(np.float16)
    masks_by_par = [_host_masks(0), _host_masks(1)]

    in_maps = []
    for c in range(8):
        b, par = c // 2, c % 2
        own = OWN[par]
        xTb = np.ascontiguousarray(x[b].T.astype(np.float16))
        cols = np.concatenate([np.arange(128 * t, 128 * (t + 1)) for t in own])
        xqT = np.ascontiguousarray(xTb[:, cols])
        m256, m128 = masks_by_par[par]
        in_maps.append({"xT": xTb, "xqT": xqT, "wq": wq, "wv": wv,
                        "bq": bqv, "m256": m256, "m128": m128})

    res = run_bass_kernel_spmd(nc, in_maps, core_ids=list(range(8)))
    if _results_only:
        return res

    out = np.empty((B, T, D), np.float32)
    for c in range(8):
        b, par = c // 2, c % 2
        part = res.results[c]["out"].astype(np.float32)
        for s, t in enumerate(OWN[par]):
            out[b, 128 * t:128 * (t + 1), :] = part[128 * s:128 * (s + 1), :] + b_eff
    return out



# revision 2
# speedup vs baseline: 1.0007x; 1.0007x over previous
"""Trainium2 Bass kernel for nn_CausalSelfAttention_8237747274097 — v2.

All-fp8 DoubleRow rewrite with residual compensation.

Math (exact folds, as v1):
    qkv = x @ W_attn + b_attn ; q,k,v = split ; single-head attention.
    Wqk = (Wq/8) @ Wk^T folded (no K projection);  Wvp = Wv @ W_proj folded
    (no output projection); k-bias drops, v-bias folds into host b_eff.
    Per-key score bias t_j = (bq/8)·Wk^T·x_j enters via a K=4 init matmul.

Numerics: every matmul runs as fp8e4m3 DoubleRow (2 k-chunks per
instruction, 0.5 cycles/out-elem) with hi+lo residual compensation:
for operands A≈Ah+Al, B≈Bh+Bl the product uses 3 chains Ah·Bh + Al·Bh +
Ah·Bl accumulated in one PSUM (residuals are UNSCALED e4m3 — fp8
subnormals verified exact on HW).  exp outputs are split as e5m2 hi+lo
(ph + pl = p exactly to ~1.6%).  Verified end-to-end metric 2.8e-3 vs
the 2e-2 gate in numpy emulation.

Scales (exact pow-2): Wqk*512 (scores descaled inside exp), Wvp*64
(descaled via the den=64*sum(p) reciprocal).  Global exp shift -2.75
cancels in softmax and keeps exp(s) inside e5m2 range.

Causal masking: dead 128-col regions are killed by the t-init matmul
itself (row2 = -240 times a data-selected 240-pattern => psS <= -5e4 =>
exp == 0 exactly); diagonal tiles get a triangular multiply on the f32
exp output.  All parity differences are DATA (mov_sel / dmask), so one
NEFF serves all 8 cores.

Sharding (unchanged from v1): core c = (batch c//2, parity c%2); each
core owns 8 of 16 query row-tiles (OWN), computes full V for its batch.
"""

import numpy as np
import ml_dtypes

import concourse.bass as bass
import concourse.tile as tile
import concourse.mybir as mybir
from concourse import bacc
from concourse.bass import ts, ds
from concourse.bass_utils import run_bass_kernel_spmd

F32 = mybir.dt.float32
F16 = mybir.dt.float16
E4 = mybir.dt.float8e4
E5 = mybir.dt.float8e5
DR = mybir.MatmulPerfMode.DoubleRow
NE4 = ml_dtypes.float8_e4m3
NE5 = ml_dtypes.float8_e5m2

T, D = 2048, 1024
NT = T // 128          # 16 key/query tiles
DP = 4                 # d-chunk pairs (8 chunks of 128, DoubleRow-paired)
OWN = [[15, 12, 11, 8, 7, 4, 3, 0],
       [14, 13, 10, 9, 6, 5, 2, 1]]
CP = [16, 12, 8, 4]    # j-blocks per pair P (uniform across cores)
PAIR_ORDER = (0, 1, 2, 3)
SQ = 512.0
SV = 64.0
CSHIFT = 2.75          # exact in fp8/f32; exp(s - CSHIFT)
NBLK = sum(CP) // 2    # 20 tj-pair blocks per core

_NC_CACHE = {}


def _build(repeat=1, phases=3):
    key = (repeat, phases)
    if key in _NC_CACHE:
        return _NC_CACHE[key]
    nc = bacc.Bacc("TRN2", target_bir_lowering=False, debug=False,
                   enable_asserts=False, num_devices=8)
    t = {}
    for nm in ("xth", "xtl"):
        t[nm] = nc.dram_tensor(nm, [DP, 128, 2, T], E4, kind="ExternalInput").ap()
    for nm in ("xqh", "xql", "wqh", "wql", "wvh", "wvl"):
        t[nm] = nc.dram_tensor(nm, [DP, 128, 2, 1024], E4, kind="ExternalInput").ap()
    t["tst"] = nc.dram_tensor("tst", [2, 2, 1024], E4, kind="ExternalInput").ap()
    t["mov_sel"] = nc.dram_tensor("mov_sel", [2, NBLK, 2, 512], E4,
                                  kind="ExternalInput").ap()
    t["dmask"] = nc.dram_tensor("dmask", [2, 2, 128, 128], F32,
                                kind="ExternalInput").ap()
    t["out"] = nc.dram_tensor("out", [1024, 1024], F16, kind="ExternalOutput").ap()

    with tile.TileContext(nc, pool_alloc_mode="queue") as tc:
        def body(_i=None):
            _emit(nc, tc, t, phases)
        if repeat == 1:
            body()
        else:
            with tc.For_i(0, repeat, 1):
                body()
    nc.compile()
    _NC_CACHE[key] = nc
    return nc


def _emit(nc, tc, t, phases=3):
    with tc.tile_pool(name="xt", bufs=1) as xt_pool, \
         tc.tile_pool(name="xq", bufs=1) as xq_pool, \
         tc.tile_pool(name="wq", bufs=1) as wq_pool, \
         tc.tile_pool(name="wv", bufs=1) as wv_pool, \
         tc.tile_pool(name="gp", bufs=1) as g_pool, \
         tc.tile_pool(name="vp", bufs=1) as v_pool, \
         tc.tile_pool(name="small", bufs=1) as small:

        xth = [xt_pool.tile([128, 2, T], E4, tag=f"xth{d}", name=f"xth{d}")
               for d in range(DP)]
        xtl = [xt_pool.tile([128, 2, T], E4, tag=f"xtl{d}", name=f"xtl{d}")
               for d in range(DP)]
        xqh = [xq_pool.tile([128, 2, 1024], E4, tag=f"xqh{d}", name=f"xqh{d}")
               for d in range(DP)]
        xql = [xq_pool.tile([128, 2, 1024], E4, tag=f"xql{d}", name=f"xql{d}")
               for d in range(DP)]
        wqh = [wq_pool.tile([128, 2, 1024], E4, tag=f"wqh{d}", name=f"wqh{d}")
               for d in range(DP)]
        wql = [wq_pool.tile([128, 2, 1024], E4, tag=f"wql{d}", name=f"wql{d}")
               for d in range(DP)]
        wvh = [wv_pool.tile([128, 2, 1024], E4, tag=f"wvh{d}", name=f"wvh{d}")
               for d in range(DP)]
        wvl = [wv_pool.tile([128, 2, 1024], E4, tag=f"wvl{d}", name=f"wvl{d}")
               for d in range(DP)]
        qph = [g_pool.tile([128, 2, 1024], E4, tag=f"qph{m}", name=f"qph{m}")
               for m in range(DP)]
        qpl = [g_pool.tile([128, 2, 1024], E4, tag=f"qpl{m}", name=f"qpl{m}")
               for m in range(DP)]
        vph = [v_pool.tile([128, 2, 1024], E4, tag=f"vph{p}", name=f"vph{p}")
               for p in range(NT // 2)]
        vpl = [v_pool.tile([128, 2, 1024], E4, tag=f"vpl{p}", name=f"vpl{p}")
               for p in range(NT // 2)]
        tst = small.tile([2, 2, 1024], E4, tag="tst", name="tst_sb")
        mov = small.tile([2, NBLK, 2, 512], E4, tag="mov", name="mov_sb")
        dmsk = small.tile([128, 2, 2, 128], F32, tag="dmsk", name="dmsk_sb")
        ones64 = small.tile([128, 2, 1], E4, tag="ones", name="ones64")
        cbias = small.tile([128, 1], F32, tag="cb", name="cbias")
        nc.vector.memset(ones64[:], SV)
        nc.vector.memset(cbias[:], -CSHIFT)

        # ---- DMA choreography (SP queue is in-order) ----
        # Q cold start first, then the rest in consumption order.
        nc.sync.dma_start(wqh[0][:, :, 0:384], t["wqh"][0, :, :, 0:384])
        nc.sync.dma_start(xqh[0][:, :, 0:512], t["xqh"][0, :, :, 0:512])
        nc.sync.dma_start(xqh[0][:, :, 512:1024], t["xqh"][0, :, :, 512:1024])
        nc.sync.dma_start(wqh[0][:, :, 384:1024], t["wqh"][0, :, :, 384:1024])
        nc.sync.dma_start(wql[0][:], t["wql"][0, :, :, :])
        nc.sync.dma_start(xql[0][:], t["xql"][0, :, :, :])
        for d in range(1, DP):
            nc.sync.dma_start(wqh[d][:], t["wqh"][d, :, :, :])
            nc.sync.dma_start(xqh[d][:], t["xqh"][d, :, :, :])
            nc.sync.dma_start(wql[d][:], t["wql"][d, :, :, :])
            nc.sync.dma_start(xql[d][:], t["xql"][d, :, :, :])
        for d in range(DP):
            nc.sync.dma_start(wvh[d][:], t["wvh"][d, :, :, :])
        for d in range(DP):
            nc.sync.dma_start(xth[d][:, :, 0:1024], t["xth"][d, :, :, 0:1024])
        for d in range(DP):
            nc.sync.dma_start(wvl[d][:], t["wvl"][d, :, :, :])
        for d in range(DP):
            nc.sync.dma_start(xtl[d][:, :, 0:1024], t["xtl"][d, :, :, 0:1024])
        for d in range(DP):
            nc.sync.dma_start(xth[d][:, :, 1024:2048], t["xth"][d, :, :, 1024:2048])
            nc.sync.dma_start(xtl[d][:, :, 1024:2048], t["xtl"][d, :, :, 1024:2048])
        nc.sync.dma_start(tst[:], t["tst"][:, :, :])
        nc.sync.dma_start(mov[:], t["mov_sel"][:, :, :, :])
        # dmask dram [2,2,128,128] -> sbuf [128, 2, 2, 128]
        nc.sync.dma_start(dmsk[:], t["dmask"].rearrange("a b p c -> p a b c"))

        pt_cm = tc.tile_pool(name="ptp", bufs=1)
        trans_cm = tc.tile_pool(name="trans", bufs=1)
        psS_cm = tc.tile_pool(name="psS", bufs=1, space="PSUM")
        pt_pool = pt_cm.__enter__()
        trans = trans_cm.__enter__()
        psS_pool = psS_cm.__enter__()
        psA_cm = tc.tile_pool(name="psA", bufs=1, space="PSUM")
        psA = psA_cm.__enter__()

        # ---- Phase Q: G = x_q @ Wqk_s, stored as e4m3 hi+lo pairs ----
        CHAINS_Q = ((wqh, xqh), (wql, xqh), (wqh, xql))

        def q_copies(m, half, ps):
            mp, sub = m // 2, m % 2
            dst_h = qph[mp][:, sub, ds(512 * half, 512)]
            dst_l = qpl[mp][:, sub, ds(512 * half, 512)]
            nc.scalar.activation(dst_h, ps[:],
                                 mybir.ActivationFunctionType.Copy)
            nc.vector.tensor_sub(dst_l, ps[:], dst_h)

        # group0: dp-outer to stream arriving tiles; last dp sweep m-first
        grp = ((0, 0), (0, 1), (1, 0), (1, 1), (2, 0), (2, 1))
        pss = {mh: psA.tile([128, 512], F32, tag="A", name="psQ_t", bufs=6)
               for mh in grp}
        for dp in range(DP - 1):
            for ci, (lh, rh) in enumerate(CHAINS_Q):
                for (m, half) in grp:
                    nc.tensor.matmul(
                        pss[(m, half)][:],
                        lh[dp][:, :, ts(m, 128)],
                        rh[dp][:, :, ds(512 * half, 512)],
                        start=(dp == 0 and ci == 0),
                        stop=False,
                        perf_mode=DR)
        for (m, half) in grp:
            for ci, (lh, rh) in enumerate(CHAINS_Q):
                nc.tensor.matmul(
                    pss[(m, half)][:],
                    lh[DP - 1][:, :, ts(m, 128)],
                    rh[DP - 1][:, :, ds(512 * half, 512)],
                    start=False, stop=(ci == 2),
                    perf_mode=DR)
            q_copies(m, half, pss[(m, half)])
        # m 3..7: streamed half-tiles (all inputs resident by now)
        for m in range(3, 8):
            for half in range(2):
                ps = psA.tile([128, 512], F32, tag="A", name="psQ_t", bufs=6)
                for dp in range(DP):
                    for ci, (lh, rh) in enumerate(CHAINS_Q):
                        nc.tensor.matmul(
                            ps[:],
                            lh[dp][:, :, ts(m, 128)],
                            rh[dp][:, :, ds(512 * half, 512)],
                            start=(dp == 0 and ci == 0),
                            stop=(dp == DP - 1 and ci == 2),
                            perf_mode=DR)
                q_copies(m, half, ps)

        # ---- Phase V: VP = x @ Wvp_s (full batch), e4m3 hi+lo pairs ----
        CHAINS_V = ((xth, wvh), (xtl, wvh), (xth, wvl))
        for tt in range(NT):
            tp, sub = tt // 2, tt % 2
            for half in range(2):
                psV = psA.tile([128, 512], F32, tag="A", name="psV_t", bufs=6)
                for ci, (lh, rh) in enumerate(CHAINS_V):
                    for dp in range(DP):
                        nc.tensor.matmul(
                            psV[:],
                            lh[dp][:, :, ts(tt, 128)],
                            rh[dp][:, :, ds(512 * half, 512)],
                            start=(ci == 0 and dp == 0),
                            stop=(ci == 2 and dp == DP - 1),
                            perf_mode=DR)
                dst_h = vph[tp][:, sub, ds(512 * half, 512)]
                dst_l = vpl[tp][:, sub, ds(512 * half, 512)]
                nc.scalar.activation(dst_h, psV[:],
                                     mybir.ActivationFunctionType.Copy)
                nc.vector.tensor_sub(dst_l, psV[:], dst_h)

        if phases <= 1:
            psA_cm.__exit__(None, None, None)
            with tc.tile_pool(name="dump", bufs=1) as dump:
                tk = dump.tile([128, 512], F16, tag="tk", name="tk")
                nc.vector.tensor_copy(tk[:], vph[0][:, 0, 0:512])
                nc.sync.dma_start(t["out"][0:128, 0:512], tk[:])
            return

        # ---- Phase B: scores -> exp split -> PV + den, software-pipelined ----
        # The first pair's scores are emitted while the psA pool is still
        # open (psS takes the 2 spare PSUM banks), so phase B overlaps the V
        # tail; po/pden pools open only after psA closes.
        blk_base = {}
        acc = 0
        for P in PAIR_ORDER:
            blk_base[P] = acc
            acc += CP[P] // 2

        po_pool = pden_pool = None

        if True:
            CHAINS_S = ((xth, qph), (xtl, qph), (xth, qpl))

            def emit_scores(P):
                cp = CP[P]
                blocks = cp // 2
                pts = []
                for tjp in range(blocks):
                    bid = blk_base[P] + tjp
                    psS = psS_pool.tile([128, 2, 256], F32, tag="s",
                                        name="psS_t", bufs=2)
                    nc.tensor.matmul(psS[:, :, :], tst[:, :, ts(tjp, 128)],
                                     mov[:, bid, :, :], start=True, stop=False,
                                     perf_mode=DR)
                    wS = 128 if tjp == blocks - 1 else 256
                    for i in range(2):
                        tj = 2 * tjp + i
                        for dp in range(DP):
                            for ci, (lh, rh) in enumerate(CHAINS_S):
                                nc.tensor.matmul(
                                    psS[:, i, 0:wS],
                                    lh[dp][:, :, ts(tj, 128)],
                                    rh[dp][:, :, ds(P * 256, wS)],
                                    start=False,
                                    stop=(dp == DP - 1 and ci == 2),
                                    perf_mode=DR)
                    p32 = trans.tile([128, 2, 256], F32, tag="p32",
                                     name="p32_t", bufs=3)
                    nc.scalar.activation(p32[:, :, :], psS[:, :, :],
                                         mybir.ActivationFunctionType.Exp,
                                         bias=cbias[:, 0:1], scale=1.0 / SQ)
                    if tjp == blocks - 2:      # block A masks
                        nc.vector.tensor_mul(p32[:, 0, 128:256],
                                             p32[:, 0, 128:256],
                                             dmsk[:, 0, 0, :])
                        nc.vector.tensor_mul(p32[:, 1, 128:256],
                                             p32[:, 1, 128:256],
                                             dmsk[:, 0, 1, :])
                    if tjp == blocks - 1:      # block B masks
                        nc.vector.tensor_mul(p32[:, 0, 0:128],
                                             p32[:, 0, 0:128],
                                             dmsk[:, 1, 0, :])
                        nc.vector.tensor_mul(p32[:, 1, 0:128],
                                             p32[:, 1, 0:128],
                                             dmsk[:, 1, 1, :])
                    pth = pt_pool.tile([128, 2, 256], E5, tag=f"pth{tjp}",
                                       name="pth_t", bufs=2)
                    ptl = pt_pool.tile([128, 2, 256], E5, tag=f"ptl{tjp}",
                                       name="ptl_t", bufs=2)
                    nc.scalar.activation(pth[:, :, :], p32[:, :, :],
                                         mybir.ActivationFunctionType.Copy)
                    nc.vector.tensor_sub(ptl[:, :, :], p32[:, :, :],
                                         pth[:, :, :])
                    pts.append((pth, ptl))
                return pts

            def emit_pv(P, pts):
                if phases <= 2:
                    return
                cp = CP[P]
                blocks = cp // 2
                for slot in range(2):
                    nblk = blocks if slot == 0 else blocks - 1
                    row = 128 * (2 * P + slot)
                    pden = pden_pool.tile([128, 1], F32, tag=f"pd{slot}",
                                          name=f"pden{slot}_t", bufs=1)
                    rec = trans.tile([128, 1], F32, tag="rec", name="rec_t",
                                     bufs=4)
                    for half in range(2):
                        po = po_pool.tile([128, 512], F32, tag="po",
                                          name="po_t", bufs=2)
                        for bi in range(nblk):
                            ph, pl = pts[bi]
                            sh = ph[:, :, ds(slot * 128, 128)]
                            sl = pl[:, :, ds(slot * 128, 128)]
                            if half == 0:
                                nc.tensor.matmul(pden[:], sh, ones64[:, :, :],
                                                 start=(bi == 0), stop=False,
                                                 perf_mode=DR)
                                nc.tensor.matmul(pden[:], sl, ones64[:, :, :],
                                                 start=False,
                                                 stop=(bi == nblk - 1),
                                                 perf_mode=DR)
                            mh = vph[bi][:, :, ds(512 * half, 512)]
                            ml = vpl[bi][:, :, ds(512 * half, 512)]
                            for ci, (s_, m_) in enumerate(
                                    ((sh, mh), (sl, mh), (sh, ml))):
                                nc.tensor.matmul(
                                    po[:], s_, m_,
                                    start=(bi == 0 and ci == 0),
                                    stop=(bi == nblk - 1 and ci == 2),
                                    perf_mode=DR)
                        if half == 0:
                            nc.vector.reciprocal(rec[:], pden[:])
                            ob = trans.tile([128, 512], F16, tag="ob",
                                            name="ob_t", bufs=4)
                            nc.scalar.activation(
                                ob[:], po[:],
                                mybir.ActivationFunctionType.Copy,
                                bias=0.0, scale=rec[:, 0:1])
                        else:
                            ob = trans.tile([128, 512], F16, tag="ob",
                                            name="ob_t", bufs=4)
                            nc.vector.tensor_scalar_mul(ob[:], po[:],
                                                        rec[:, 0:1])
                        nc.sync.dma_start(
                            t["out"][ds(row, 128), ds(512 * half, 512)],
                            ob[:])

            prev = None
            for P in PAIR_ORDER:
                pts = emit_scores(P)
                if po_pool is None:
                    # first pair's scores emitted; now V is long done --
                    # release psA and claim its banks for PV accumulators
                    psA_cm.__exit__(None, None, None)
                    po_cm = tc.tile_pool(name="poP", bufs=1, space="PSUM")
                    pden_cm = tc.tile_pool(name="pdn", bufs=1, space="PSUM")
                    po_pool = po_cm.__enter__()
                    pden_pool = pden_cm.__enter__()
                if prev is not None:
                    emit_pv(*prev)
                prev = (P, pts)
            emit_pv(*prev)
            pden_cm.__exit__(None, None, None)
            po_cm.__exit__(None, None, None)
        psS_cm.__exit__(None, None, None)
        trans_cm.__exit__(None, None, None)
        pt_cm.__exit__(None, None, None)


def _split8(a):
    """f32 array -> (hi, lo) e4m3 pair with unscaled residual."""
    hi = a.astype(NE4)
    lo = (a - hi.astype(np.float32)).astype(NE4)
    return hi, lo


def _wlayout(w):
    # [1024 (d), 1024 (m)] -> [DP, 128, 2, 1024]: pair d-chunks for DoubleRow
    return np.ascontiguousarray(
        w.reshape(DP, 2, 128, 1024).transpose(0, 2, 1, 3))


def _host_consts(par):
    """mov_sel (t-init moving patterns) and dmask for a parity."""
    v = np.zeros((4, 4, 512), np.float32)
    for k in range(4):
        v[k, 0, 0:256] = 1.0
        v[k, 1, 256:512] = 1.0
    v[1, 2, 384:512] = 240.0                      # parity0 block A
    v[2, 2, 128:256] = 240.0                      # parity0 block B
    v[2, 2, 384:512] = 240.0
    v[3, 2, 128:512] = 240.0                      # parity1 block B
    mov = np.zeros((NBLK, 4, 512), np.float32)
    bi = 0
    for P in PAIR_ORDER:
        blocks = CP[P] // 2
        for tjp in range(blocks):
            if par == 0 and tjp == blocks - 2:
                sel = 1
            elif par == 0 and tjp == blocks - 1:
                sel = 2
            elif par == 1 and tjp == blocks - 1:
                sel = 3
            else:
                sel = 0
            mov[bi] = v[sel]
            bi += 1
    assert bi == NBLK
    # DoubleRow layout [2(k), NBLK, 2(i), 512]: k0i0=row0, k1i0=row2(kill),
    # k0i1=row1, k1i1=0
    mov_dr = np.zeros((2, NBLK, 2, 512), np.float32)
    mov_dr[0, :, 0, :] = mov[:, 0, :]
    mov_dr[1, :, 0, :] = mov[:, 2, :]
    mov_dr[0, :, 1, :] = mov[:, 1, :]
    mov = mov_dr

    tri = (np.arange(128)[:, None] <= np.arange(128)[None, :]).astype(np.float32)
    ones = np.ones((128, 128), np.float32)
    dm = np.empty((2, 2, 128, 128), np.float32)
    if par == 0:
        dm[0, 0], dm[0, 1] = tri, ones            # block A: diag at [i=0,slot1]
        dm[1, 0], dm[1, 1] = ones, tri            # block B: diag at [i=1,slot0]
    else:
        dm[0, 0], dm[0, 1] = ones, tri            # block A: diag at [i=1,slot1]
        dm[1, 0], dm[1, 1] = tri, ones            # block B: diag at [i=0,slot0]
    return mov.astype(NE4), dm


def kernel(x, W_attn, b_attn, W_proj, b_proj, _repeat=1, _results_only=False,
           _phases=3):
    x = np.asarray(x, np.float32)
    W_attn = np.asarray(W_attn, np.float64)
    b_attn = np.asarray(b_attn, np.float64)
    W_proj = np.asarray(W_proj, np.float64)
    b_proj = np.asarray(b_proj, np.float64)
    B = x.shape[0]

    nc = _build(_repeat, _phases)

    Wq = W_attn[:, :D]
    Wk = W_attn[:, D:2 * D]
    Wv = W_attn[:, 2 * D:]
    wqk_s = np.float32((Wq * 0.125) @ Wk.T * SQ)
    wvp_s = np.float32(Wv @ W_proj * SV)
    bqk = (b_attn[:D] * 0.125) @ Wk.T            # [D], f64
    b_eff = np.float32(b_proj + b_attn[2 * D:] @ W_proj)

    wqh, wql = _split8(wqk_s)
    wvh, wvl = _split8(wvp_s)
    wqh, wql = _wlayout(wqh), _wlayout(wql)
    wvh, wvl = _wlayout(wvh), _wlayout(wvl)
    consts = [_host_consts(0), _host_consts(1)]

    def xlayout(a, n):
        # [1024 (d), n (tok)] e4 -> [DP, 128, 2, n]
        return np.ascontiguousarray(
            a.reshape(DP, 2, 128, n).transpose(0, 2, 1, 3))

    in_maps = []
    for c in range(8):
        b, par = c // 2, c % 2
        own = OWN[par]
        xTb = np.ascontiguousarray(x[b].T)       # [D, T] f32
        xh, xl = _split8(xTb)
        cols = np.concatenate([np.arange(128 * q, 128 * (q + 1)) for q in own])
        tvec = np.float32(x[b].astype(np.float64) @ bqk) * np.float32(SQ)
        trows = tvec.reshape(8, 2, 128).transpose(1, 0, 2).reshape(2, 1024)
        tst = np.zeros((2, 2, 1024), np.float32)
        tst[0, 0] = trows[0]        # k0,i0: t(tj0)
        tst[1, 0] = -240.0          # k1,i0: kill row
        tst[0, 1] = trows[1]        # k0,i1: t(tj1)
        mov, dm = consts[par]
        in_maps.append({
            "xth": xlayout(xh, T), "xtl": xlayout(xl, T),
            "xqh": xlayout(np.ascontiguousarray(xh[:, cols]), 1024),
            "xql": xlayout(np.ascontiguousarray(xl[:, cols]), 1024),
            "wqh": wqh, "wql": wql, "wvh": wvh, "wvl": wvl,
            "tst": tst.astype(NE4), "mov_sel": mov, "dmask": dm,
        })

    res = run_bass_kernel_spmd(nc, in_maps, core_ids=list(range(8)))
    if _results_only:
        return res

    out = np.empty((B, T, D), np.float32)
    for c in range(8):
        b, par = c // 2, c % 2
        part = res.results[c]["out"].astype(np.float32)
        for s, q in enumerate(OWN[par]):
            out[b, 128 * q:128 * (q + 1), :] = part[128 * s:128 * (s + 1), :] + b_eff
    return out


# revision 3
# speedup vs baseline: 1.0013x; 1.0006x over previous
"""Trainium2 Bass kernel for nn_CausalSelfAttention_8237747274097 — v2.

All-fp8 DoubleRow rewrite with residual compensation.

Math (exact folds, as v1):
    qkv = x @ W_attn + b_attn ; q,k,v = split ; single-head attention.
    Wqk = (Wq/8) @ Wk^T folded (no K projection);  Wvp = Wv @ W_proj folded
    (no output projection); k-bias drops, v-bias folds into host b_eff.
    Per-key score bias t_j = (bq/8)·Wk^T·x_j enters via a K=4 init matmul.

Numerics: every matmul runs as fp8e4m3 DoubleRow (2 k-chunks per
instruction, 0.5 cycles/out-elem) with hi+lo residual compensation:
for operands A≈Ah+Al, B≈Bh+Bl the product uses 3 chains Ah·Bh + Al·Bh +
Ah·Bl accumulated in one PSUM (residuals are UNSCALED e4m3 — fp8
subnormals verified exact on HW).  exp outputs are split as e5m2 hi+lo
(ph + pl = p exactly to ~1.6%).  Verified end-to-end metric 2.8e-3 vs
the 2e-2 gate in numpy emulation.

Scales (exact pow-2): Wqk*512 (scores descaled inside exp), Wvp*64
(descaled via the den=64*sum(p) reciprocal).  Global exp shift -2.75
cancels in softmax and keeps exp(s) inside e5m2 range.

Causal masking: dead 128-col regions are killed by the t-init matmul
itself (row2 = -240 times a data-selected 240-pattern => psS <= -5e4 =>
exp == 0 exactly); diagonal tiles get a triangular multiply on the f32
exp output.  All parity differences are DATA (mov_sel / dmask), so one
NEFF serves all 8 cores.

Sharding (unchanged from v1): core c = (batch c//2, parity c%2); each
core owns 8 of 16 query row-tiles (OWN), computes full V for its batch.
"""

import numpy as np
import ml_dtypes

import concourse.bass as bass
import concourse.tile as tile
import concourse.mybir as mybir
from concourse import bacc
from concourse.bass import ts, ds
from concourse.bass_utils import run_bass_kernel_spmd

F32 = mybir.dt.float32
F16 = mybir.dt.float16
E4 = mybir.dt.float8e4
E5 = mybir.dt.float8e5
DR = mybir.MatmulPerfMode.DoubleRow
NE4 = ml_dtypes.float8_e4m3
NE5 = ml_dtypes.float8_e5m2

T, D = 2048, 1024
NT = T // 128          # 16 key/query tiles
DP = 4                 # d-chunk pairs (8 chunks of 128, DoubleRow-paired)
OWN = [[15, 12, 11, 8, 7, 4, 3, 0],
       [14, 13, 10, 9, 6, 5, 2, 1]]
CP = [16, 12, 8, 4]    # j-blocks per pair P (uniform across cores)
PAIR_ORDER = (0, 1, 2, 3)
SQ = 512.0
SV = 64.0
CSHIFT = 2.75          # exact in fp8/f32; exp(s - CSHIFT)
NBLK = sum(CP) // 2    # 20 tj-pair blocks per core

_NC_CACHE = {}


def _build(repeat=1, phases=3):
    key = (repeat, phases)
    if key in _NC_CACHE:
        return _NC_CACHE[key]
    nc = bacc.Bacc("TRN2", target_bir_lowering=False, debug=False,
                   enable_asserts=False, num_devices=8)
    t = {}
    for nm in ("xth", "xtl"):
        t[nm] = nc.dram_tensor(nm, [DP, 128, 2, T], E4, kind="ExternalInput").ap()
    for nm in ("xqh", "xql", "wqh", "wql", "wvh", "wvl"):
        t[nm] = nc.dram_tensor(nm, [DP, 128, 2, 1024], E4, kind="ExternalInput").ap()
    t["tst"] = nc.dram_tensor("tst", [2, 2, 1024], E4, kind="ExternalInput").ap()
    t["mov_sel"] = nc.dram_tensor("mov_sel", [2, NBLK, 2, 512], E4,
                                  kind="ExternalInput").ap()
    t["dmask"] = nc.dram_tensor("dmask", [2, 2, 128, 128], F32,
                                kind="ExternalInput").ap()
    t["out"] = nc.dram_tensor("out", [1024, 1024], F16, kind="ExternalOutput").ap()

    with tile.TileContext(nc, pool_alloc_mode="queue") as tc:
        def body(_i=None):
            _emit(nc, tc, t, phases)
        if repeat == 1:
            body()
        else:
            with tc.For_i(0, repeat, 1):
                body()
    nc.compile()
    _NC_CACHE[key] = nc
    return nc


def _emit(nc, tc, t, phases=3):
    with tc.tile_pool(name="xt", bufs=1) as xt_pool, \
         tc.tile_pool(name="xq", bufs=1) as xq_pool, \
         tc.tile_pool(name="wq", bufs=1) as wq_pool, \
         tc.tile_pool(name="wv", bufs=1) as wv_pool, \
         tc.tile_pool(name="gp", bufs=1) as g_pool, \
         tc.tile_pool(name="vp", bufs=1) as v_pool, \
         tc.tile_pool(name="small", bufs=1) as small:

        xth = [xt_pool.tile([128, 2, T], E4, tag=f"xth{d}", name=f"xth{d}")
               for d in range(DP)]
        xtl = [xt_pool.tile([128, 2, T], E4, tag=f"xtl{d}", name=f"xtl{d}")
               for d in range(DP)]
        xqh = [xq_pool.tile([128, 2, 1024], E4, tag=f"xqh{d}", name=f"xqh{d}")
               for d in range(DP)]
        xql = [xq_pool.tile([128, 2, 1024], E4, tag=f"xql{d}", name=f"xql{d}")
               for d in range(DP)]
        wqh = [wq_pool.tile([128, 2, 1024], E4, tag=f"wqh{d}", name=f"wqh{d}")
               for d in range(DP)]
        wql = [wq_pool.tile([128, 2, 1024], E4, tag=f"wql{d}", name=f"wql{d}")
               for d in range(DP)]
        wvh = [wv_pool.tile([128, 2, 1024], E4, tag=f"wvh{d}", name=f"wvh{d}")
               for d in range(DP)]
        wvl = [wv_pool.tile([128, 2, 1024], E4, tag=f"wvl{d}", name=f"wvl{d}")
               for d in range(DP)]
        qph = [g_pool.tile([128, 2, 1024], E4, tag=f"qph{m}", name=f"qph{m}")
               for m in range(DP)]
        qpl = [g_pool.tile([128, 2, 1024], E4, tag=f"qpl{m}", name=f"qpl{m}")
               for m in range(DP)]
        vph = [v_pool.tile([128, 2, 1024], E4, tag=f"vph{p}", name=f"vph{p}")
               for p in range(NT // 2)]
        vpl = [v_pool.tile([128, 2, 1024], E4, tag=f"vpl{p}", name=f"vpl{p}")
               for p in range(NT // 2)]
        tst = small.tile([2, 2, 1024], E4, tag="tst", name="tst_sb")
        mov = small.tile([2, NBLK, 2, 512], E4, tag="mov", name="mov_sb")
        dmsk = small.tile([128, 2, 2, 128], F32, tag="dmsk", name="dmsk_sb")
        ones64 = small.tile([128, 2, 1], E4, tag="ones", name="ones64")
        cbias = small.tile([128, 1], F32, tag="cb", name="cbias")
        nc.vector.memset(ones64[:], SV)
        nc.vector.memset(cbias[:], -CSHIFT)

        # ---- DMA choreography (SP queue is in-order) ----
        # Q cold start first, then the rest in consumption order.
        nc.sync.dma_start(wqh[0][:, :, 0:384], t["wqh"][0, :, :, 0:384])
        nc.sync.dma_start(xqh[0][:, :, 0:512], t["xqh"][0, :, :, 0:512])
        nc.sync.dma_start(xqh[0][:, :, 512:1024], t["xqh"][0, :, :, 512:1024])
        nc.sync.dma_start(wqh[0][:, :, 384:1024], t["wqh"][0, :, :, 384:1024])
        nc.sync.dma_start(wql[0][:], t["wql"][0, :, :, :])
        nc.sync.dma_start(xql[0][:], t["xql"][0, :, :, :])
        for d in range(1, DP):
            nc.sync.dma_start(wqh[d][:], t["wqh"][d, :, :, :])
            nc.sync.dma_start(xqh[d][:], t["xqh"][d, :, :, :])
            nc.sync.dma_start(wql[d][:], t["wql"][d, :, :, :])
            nc.sync.dma_start(xql[d][:], t["xql"][d, :, :, :])
        for d in range(DP):
            nc.sync.dma_start(wvh[d][:], t["wvh"][d, :, :, :])
        for d in range(DP):
            nc.sync.dma_start(xth[d][:, :, 0:1024], t["xth"][d, :, :, 0:1024])
        for d in range(DP):
            nc.sync.dma_start(wvl[d][:], t["wvl"][d, :, :, :])
        for d in range(DP):
            nc.sync.dma_start(xtl[d][:, :, 0:1024], t["xtl"][d, :, :, 0:1024])
        for d in range(DP):
            nc.sync.dma_start(xth[d][:, :, 1024:2048], t["xth"][d, :, :, 1024:2048])
            nc.sync.dma_start(xtl[d][:, :, 1024:2048], t["xtl"][d, :, :, 1024:2048])
        nc.sync.dma_start(tst[:], t["tst"][:, :, :])
        nc.sync.dma_start(mov[:], t["mov_sel"][:, :, :, :])
        # dmask dram [2,2,128,128] -> sbuf [128, 2, 2, 128]
        nc.sync.dma_start(dmsk[:], t["dmask"].rearrange("a b p c -> p a b c"))

        pt_cm = tc.tile_pool(name="ptp", bufs=1)
        trans_cm = tc.tile_pool(name="trans", bufs=1)
        psS_cm = tc.tile_pool(name="psS", bufs=1, space="PSUM")
        pt_pool = pt_cm.__enter__()
        trans = trans_cm.__enter__()
        psS_pool = psS_cm.__enter__()
        psA_cm = tc.tile_pool(name="psA", bufs=1, space="PSUM")
        psA = psA_cm.__enter__()

        # ---- Phase Q: G = x_q @ Wqk_s, stored as e4m3 hi+lo pairs ----
        CHAINS_Q = ((wqh, xqh), (wql, xqh), (wqh, xql))

        def q_copies(m, half, ps):
            mp, sub = m // 2, m % 2
            dst_h = qph[mp][:, sub, ds(512 * half, 512)]
            dst_l = qpl[mp][:, sub, ds(512 * half, 512)]
            nc.scalar.activation(dst_h, ps[:],
                                 mybir.ActivationFunctionType.Copy)
            nc.vector.tensor_sub(dst_l, ps[:], dst_h)

        # group0: dp-outer to stream arriving tiles; last dp sweep m-first
        grp = ((0, 0), (0, 1), (1, 0), (1, 1), (2, 0), (2, 1))
        pss = {mh: psA.tile([128, 512], F32, tag="A", name="psQ_t", bufs=6)
               for mh in grp}
        for dp in range(DP - 1):
            for ci, (lh, rh) in enumerate(CHAINS_Q):
                for (m, half) in grp:
                    nc.tensor.matmul(
                        pss[(m, half)][:],
                        lh[dp][:, :, ts(m, 128)],
                        rh[dp][:, :, ds(512 * half, 512)],
                        start=(dp == 0 and ci == 0),
                        stop=False,
                        perf_mode=DR)
        for (m, half) in grp:
            for ci, (lh, rh) in enumerate(CHAINS_Q):
                nc.tensor.matmul(
                    pss[(m, half)][:],
                    lh[DP - 1][:, :, ts(m, 128)],
                    rh[DP - 1][:, :, ds(512 * half, 512)],
                    start=False, stop=(ci == 2),
                    perf_mode=DR)
            q_copies(m, half, pss[(m, half)])
        # m 3..7: streamed half-tiles (all inputs resident by now)
        for m in range(3, 8):
            for half in range(2):
                ps = psA.tile([128, 512], F32, tag="A", name="psQ_t", bufs=6)
                for dp in range(DP):
                    for ci, (lh, rh) in enumerate(CHAINS_Q):
                        nc.tensor.matmul(
                            ps[:],
                            lh[dp][:, :, ts(m, 128)],
                            rh[dp][:, :, ds(512 * half, 512)],
                            start=(dp == 0 and ci == 0),
                            stop=(dp == DP - 1 and ci == 2),
                            perf_mode=DR)
                q_copies(m, half, ps)

        # ---- Phase V: VP = x @ Wvp_s (full batch), e4m3 hi+lo pairs ----
        CHAINS_V = ((xth, wvh), (xtl, wvh), (xth, wvl))
        for tt in range(NT):
            tp, sub = tt // 2, tt % 2
            for half in range(2):
                psV = psA.tile([128, 512], F32, tag="A", name="psV_t", bufs=6)
                for ci, (lh, rh) in enumerate(CHAINS_V):
                    for dp in range(DP):
                        nc.tensor.matmul(
                            psV[:],
                            lh[dp][:, :, ts(tt, 128)],
                            rh[dp][:, :, ds(512 * half, 512)],
                            start=(ci == 0 and dp == 0),
                            stop=(ci == 2 and dp == DP - 1),
                            perf_mode=DR)
                dst_h = vph[tp][:, sub, ds(512 * half, 512)]
                dst_l = vpl[tp][:, sub, ds(512 * half, 512)]
                nc.scalar.activation(dst_h, psV[:],
                                     mybir.ActivationFunctionType.Copy)
                nc.vector.tensor_sub(dst_l, psV[:], dst_h)

        if phases <= 1:
            psA_cm.__exit__(None, None, None)
            with tc.tile_pool(name="dump", bufs=1) as dump:
                tk = dump.tile([128, 512], F16, tag="tk", name="tk")
                nc.vector.tensor_copy(tk[:], vph[0][:, 0, 0:512])
                nc.sync.dma_start(t["out"][0:128, 0:512], tk[:])
            return

        # ---- Phase B: scores -> exp split -> PV + den, software-pipelined ----
        # The first pair's scores are emitted while the psA pool is still
        # open (psS takes the 2 spare PSUM banks), so phase B overlaps the V
        # tail; po/pden pools open only after psA closes.
        blk_base = {}
        acc = 0
        for P in PAIR_ORDER:
            blk_base[P] = acc
            acc += CP[P] // 2

        po_pool = pden_pool = None

        if True:
            CHAINS_S = ((xth, qph), (xtl, qph), (xth, qpl))

            def emit_scores(P):
                cp = CP[P]
                blocks = cp // 2
                pts = []
                for tjp in range(blocks):
                    bid = blk_base[P] + tjp
                    psS = psS_pool.tile([128, 2, 256], F32, tag="s",
                                        name="psS_t", bufs=2)
                    nc.tensor.matmul(psS[:, :, :], tst[:, :, ts(tjp, 128)],
                                     mov[:, bid, :, :], start=True, stop=False,
                                     perf_mode=DR)
                    wS = 128 if tjp == blocks - 1 else 256
                    for i in range(2):
                        tj = 2 * tjp + i
                        for dp in range(DP):
                            for ci, (lh, rh) in enumerate(CHAINS_S):
                                nc.tensor.matmul(
                                    psS[:, i, 0:wS],
                                    lh[dp][:, :, ts(tj, 128)],
                                    rh[dp][:, :, ds(P * 256, wS)],
                                    start=False,
                                    stop=(dp == DP - 1 and ci == 2),
                                    perf_mode=DR)
                    p32 = trans.tile([128, 2, 256], F32, tag="p32",
                                     name="p32_t", bufs=3)
                    nc.scalar.activation(p32[:, :, :], psS[:, :, :],
                                         mybir.ActivationFunctionType.Exp,
                                         bias=cbias[:, 0:1], scale=1.0 / SQ)
                    if tjp == blocks - 2:      # block A masks
                        nc.vector.tensor_mul(p32[:, 0, 128:256],
                                             p32[:, 0, 128:256],
                                             dmsk[:, 0, 0, :])
                        nc.vector.tensor_mul(p32[:, 1, 128:256],
                                             p32[:, 1, 128:256],
                                             dmsk[:, 0, 1, :])
                    if tjp == blocks - 1:      # block B masks
                        nc.vector.tensor_mul(p32[:, 0, 0:128],
                                             p32[:, 0, 0:128],
                                             dmsk[:, 1, 0, :])
                        nc.vector.tensor_mul(p32[:, 1, 0:128],
                                             p32[:, 1, 0:128],
                                             dmsk[:, 1, 1, :])
                    pth = pt_pool.tile([128, 2, 256], E5, tag=f"pth{tjp}",
                                       name="pth_t", bufs=2)
                    ptl = pt_pool.tile([128, 2, 256], E5, tag=f"ptl{tjp}",
                                       name="ptl_t", bufs=2)
                    nc.scalar.activation(pth[:, :, :], p32[:, :, :],
                                         mybir.ActivationFunctionType.Copy)
                    nc.vector.tensor_sub(ptl[:, :, :], p32[:, :, :],
                                         pth[:, :, :])
                    pts.append((pth, ptl))
                return pts

            def emit_pv(P, pts):
                if phases <= 2:
                    return
                cp = CP[P]
                blocks = cp // 2
                for slot in range(2):
                    nblk = blocks if slot == 0 else blocks - 1
                    row = 128 * (2 * P + slot)
                    pden = pden_pool.tile([128, 1], F32, tag=f"pd{slot}",
                                          name=f"pden{slot}_t", bufs=1)
                    rec = trans.tile([128, 1], F32, tag="rec", name="rec_t",
                                     bufs=4)
                    ob = trans.tile([128, 1024], F16, tag="ob",
                                    name="ob_t", bufs=3)
                    for half in range(2):
                        po = po_pool.tile([128, 512], F32, tag="po",
                                          name="po_t", bufs=2)
                        for bi in range(nblk):
                            ph, pl = pts[bi]
                            sh = ph[:, :, ds(slot * 128, 128)]
                            sl = pl[:, :, ds(slot * 128, 128)]
                            if half == 0:
                                nc.tensor.matmul(pden[:], sh, ones64[:, :, :],
                                                 start=(bi == 0), stop=False,
                                                 perf_mode=DR)
                                nc.tensor.matmul(pden[:], sl, ones64[:, :, :],
                                                 start=False,
                                                 stop=(bi == nblk - 1),
                                                 perf_mode=DR)
                            mh = vph[bi][:, :, ds(512 * half, 512)]
                            ml = vpl[bi][:, :, ds(512 * half, 512)]
                            for ci, (s_, m_) in enumerate(
                                    ((sh, mh), (sl, mh), (sh, ml))):
                                nc.tensor.matmul(
                                    po[:], s_, m_,
                                    start=(bi == 0 and ci == 0),
                                    stop=(bi == nblk - 1 and ci == 2),
                                    perf_mode=DR)
                        if half == 0:
                            nc.vector.reciprocal(rec[:], pden[:])
                            nc.scalar.activation(
                                ob[:, 0:512], po[:],
                                mybir.ActivationFunctionType.Copy,
                                bias=0.0, scale=rec[:, 0:1])
                        else:
                            nc.vector.tensor_scalar_mul(ob[:, 512:1024], po[:],
                                                        rec[:, 0:1])
                    nc.sync.dma_start(t["out"][ds(row, 128), :], ob[:])

            prev = None
            for P in PAIR_ORDER:
                pts = emit_scores(P)
                if po_pool is None:
                    # first pair's scores emitted; now V is long done --
                    # release psA and claim its banks for PV accumulators
                    psA_cm.__exit__(None, None, None)
                    po_cm = tc.tile_pool(name="poP", bufs=1, space="PSUM")
                    pden_cm = tc.tile_pool(name="pdn", bufs=1, space="PSUM")
                    po_pool = po_cm.__enter__()
                    pden_pool = pden_cm.__enter__()
                if prev is not None:
                    emit_pv(*prev)
                prev = (P, pts)
            emit_pv(*prev)
            pden_cm.__exit__(None, None, None)
            po_cm.__exit__(None, None, None)
        psS_cm.__exit__(None, None, None)
        trans_cm.__exit__(None, None, None)
        pt_cm.__exit__(None, None, None)


def _split8(a):
    """f32 array -> (hi, lo) e4m3 pair with unscaled residual."""
    hi = a.astype(NE4)
    lo = (a - hi.astype(np.float32)).astype(NE4)
    return hi, lo


def _wlayout(w):
    # [1024 (d), 1024 (m)] -> [DP, 128, 2, 1024]: pair d-chunks for DoubleRow
    return np.ascontiguousarray(
        w.reshape(DP, 2, 128, 1024).transpose(0, 2, 1, 3))


def _host_consts(par):
    """mov_sel (t-init moving patterns) and dmask for a parity."""
    v = np.zeros((4, 4, 512), np.float32)
    for k in range(4):
        v[k, 0, 0:256] = 1.0
        v[k, 1, 256:512] = 1.0
    v[1, 2, 384:512] = 240.0                      # parity0 block A
    v[2, 2, 128:256] = 240.0                      # parity0 block B
    v[2, 2, 384:512] = 240.0
    v[3, 2, 128:512] = 240.0                      # parity1 block B
    mov = np.zeros((NBLK, 4, 512), np.float32)
    bi = 0
    for P in PAIR_ORDER:
        blocks = CP[P] // 2
        for tjp in range(blocks):
            if par == 0 and tjp == blocks - 2:
                sel = 1
            elif par == 0 and tjp == blocks - 1:
                sel = 2
            elif par == 1 and tjp == blocks - 1:
                sel = 3
            else:
                sel = 0
            mov[bi] = v[sel]
            bi += 1
    assert bi == NBLK
    # DoubleRow layout [2(k), NBLK, 2(i), 512]: k0i0=row0, k1i0=row2(kill),
    # k0i1=row1, k1i1=0
    mov_dr = np.zeros((2, NBLK, 2, 512), np.float32)
    mov_dr[0, :, 0, :] = mov[:, 0, :]
    mov_dr[1, :, 0, :] = mov[:, 2, :]
    mov_dr[0, :, 1, :] = mov[:, 1, :]
    mov = mov_dr

    tri = (np.arange(128)[:, None] <= np.arange(128)[None, :]).astype(np.float32)
    ones = np.ones((128, 128), np.float32)
    dm = np.empty((2, 2, 128, 128), np.float32)
    if par == 0:
        dm[0, 0], dm[0, 1] = tri, ones            # block A: diag at [i=0,slot1]
        dm[1, 0], dm[1, 1] = ones, tri            # block B: diag at [i=1,slot0]
    else:
        dm[0, 0], dm[0, 1] = ones, tri            # block A: diag at [i=1,slot1]
        dm[1, 0], dm[1, 1] = tri, ones            # block B: diag at [i=0,slot0]
    return mov.astype(NE4), dm


def kernel(x, W_attn, b_attn, W_proj, b_proj, _repeat=1, _results_only=False,
           _phases=3):
    x = np.asarray(x, np.float32)
    W_attn = np.asarray(W_attn, np.float64)
    b_attn = np.asarray(b_attn, np.float64)
    W_proj = np.asarray(W_proj, np.float64)
    b_proj = np.asarray(b_proj, np.float64)
    B = x.shape[0]

    nc = _build(_repeat, _phases)

    Wq = W_attn[:, :D]
    Wk = W_attn[:, D:2 * D]
    Wv = W_attn[:, 2 * D:]
    wqk_s = np.float32((Wq * 0.125) @ Wk.T * SQ)
    wvp_s = np.float32(Wv @ W_proj * SV)
    bqk = (b_attn[:D] * 0.125) @ Wk.T            # [D], f64
    b_eff = np.float32(b_proj + b_attn[2 * D:] @ W_proj)

    wqh, wql = _split8(wqk_s)
    wvh, wvl = _split8(wvp_s)
    wqh, wql = _wlayout(wqh), _wlayout(wql)
    wvh, wvl = _wlayout(wvh), _wlayout(wvl)
    consts = [_host_consts(0), _host_consts(1)]

    def xlayout(a, n):
        # [1024 (d), n (tok)] e4 -> [DP, 128, 2, n]
        return np.ascontiguousarray(
            a.reshape(DP, 2, 128, n).transpose(0, 2, 1, 3))

    in_maps = []
    for c in range(8):
        b, par = c // 2, c % 2
        own = OWN[par]
        xTb = np.ascontiguousarray(x[b].T)       # [D, T] f32
        xh, xl = _split8(xTb)
        cols = np.concatenate([np.arange(128 * q, 128 * (q + 1)) for q in own])
        tvec = np.float32(x[b].astype(np.float64) @ bqk) * np.float32(SQ)
        trows = tvec.reshape(8, 2, 128).transpose(1, 0, 2).reshape(2, 1024)
        tst = np.zeros((2, 2, 1024), np.float32)
        tst[0, 0] = trows[0]        # k0,i0: t(tj0)
        tst[1, 0] = -240.0          # k1,i0: kill row
        tst[0, 1] = trows[1]        # k0,i1: t(tj1)
        mov, dm = consts[par]
        in_maps.append({
            "xth": xlayout(xh, T), "xtl": xlayout(xl, T),
            "xqh": xlayout(np.ascontiguousarray(xh[:, cols]), 1024),
            "xql": xlayout(np.ascontiguousarray(xl[:, cols]), 1024),
            "wqh": wqh, "wql": wql, "wvh": wvh, "wvl": wvl,
            "tst": tst.astype(NE4), "mov_sel": mov, "dmask": dm,
        })

    res = run_bass_kernel_spmd(nc, in_maps, core_ids=list(range(8)))
    if _results_only:
        return res

    out = np.empty((B, T, D), np.float32)
    for c in range(8):
        b, par = c // 2, c % 2
        part = res.results[c]["out"].astype(np.float32)
        for s, q in enumerate(OWN[par]):
            out[b, 128 * q:128 * (q + 1), :] = part[128 * s:128 * (s + 1), :] + b_eff
    return out


# revision 4
# speedup vs baseline: 1.0042x; 1.0029x over previous
"""Trainium2 Bass kernel for nn_CausalSelfAttention_8237747274097 — v2.

All-fp8 DoubleRow rewrite with residual compensation.

Math (exact folds, as v1):
    qkv = x @ W_attn + b_attn ; q,k,v = split ; single-head attention.
    Wqk = (Wq/8) @ Wk^T folded (no K projection);  Wvp = Wv @ W_proj folded
    (no output projection); k-bias drops, v-bias folds into host b_eff.
    Per-key score bias t_j = (bq/8)·Wk^T·x_j enters via a K=4 init matmul.

Numerics: every matmul runs as fp8e4m3 DoubleRow (2 k-chunks per
instruction, 0.5 cycles/out-elem) with hi+lo residual compensation:
for operands A≈Ah+Al, B≈Bh+Bl the product uses 3 chains Ah·Bh + Al·Bh +
Ah·Bl accumulated in one PSUM (residuals are UNSCALED e4m3 — fp8
subnormals verified exact on HW).  exp outputs are split as e5m2 hi+lo
(ph + pl = p exactly to ~1.6%).  Verified end-to-end metric 2.8e-3 vs
the 2e-2 gate in numpy emulation.

Scales (exact pow-2): Wqk*512 (scores descaled inside exp), Wvp*64
(descaled via the den=64*sum(p) reciprocal).  Global exp shift -2.75
cancels in softmax and keeps exp(s) inside e5m2 range.

Causal masking: dead 128-col regions are killed by the t-init matmul
itself (row2 = -240 times a data-selected 240-pattern => psS <= -5e4 =>
exp == 0 exactly); diagonal tiles get a triangular multiply on the f32
exp output.  All parity differences are DATA (mov_sel / dmask), so one
NEFF serves all 8 cores.

Sharding (unchanged from v1): core c = (batch c//2, parity c%2); each
core owns 8 of 16 query row-tiles (OWN), computes full V for its batch.
"""

import numpy as np
import ml_dtypes

import concourse.bass as bass
import concourse.tile as tile
import concourse.mybir as mybir
from concourse import bacc
from concourse.bass import ts, ds
from concourse.bass_utils import run_bass_kernel_spmd

F32 = mybir.dt.float32
F16 = mybir.dt.float16
E4 = mybir.dt.float8e4
E5 = mybir.dt.float8e5
DR = mybir.MatmulPerfMode.DoubleRow
NE4 = ml_dtypes.float8_e4m3
NE5 = ml_dtypes.float8_e5m2

T, D = 2048, 1024
NT = T // 128          # 16 key/query tiles
DP = 4                 # d-chunk pairs (8 chunks of 128, DoubleRow-paired)
OWN = [[15, 12, 11, 8, 7, 4, 3, 0],
       [14, 13, 10, 9, 6, 5, 2, 1]]
CP = [16, 12, 8, 4]    # j-blocks per pair P (uniform across cores)
PAIR_ORDER = (0, 1, 2, 3)
SQ = 512.0
SV = 64.0
CSHIFT = 2.75          # exact in fp8/f32; exp(s - CSHIFT)
NBLK = sum(CP) // 2    # 20 tj-pair blocks per core

_NC_CACHE = {}


def _build(repeat=1, phases=3):
    key = (repeat, phases)
    if key in _NC_CACHE:
        return _NC_CACHE[key]
    nc = bacc.Bacc("TRN2", target_bir_lowering=False, debug=False,
                   enable_asserts=False, num_devices=8)
    t = {}
    for nm in ("xth", "xtl"):
        t[nm] = nc.dram_tensor(nm, [DP, 128, 2, T], E4, kind="ExternalInput").ap()
    for nm in ("xqh", "xql", "wqh", "wql", "wvh", "wvl"):
        t[nm] = nc.dram_tensor(nm, [DP, 128, 2, 1024], E4, kind="ExternalInput").ap()
    t["tst"] = nc.dram_tensor("tst", [2, 2, 1024], E4, kind="ExternalInput").ap()
    t["mov_sel"] = nc.dram_tensor("mov_sel", [2, NBLK, 2, 512], E4,
                                  kind="ExternalInput").ap()
    t["dmask"] = nc.dram_tensor("dmask", [2, 2, 128, 128], F32,
                                kind="ExternalInput").ap()
    t["out"] = nc.dram_tensor("out", [1024, 1024], F16, kind="ExternalOutput").ap()

    with tile.TileContext(nc, pool_alloc_mode="queue") as tc:
        def body(_i=None):
            _emit(nc, tc, t, phases)
        if repeat == 1:
            body()
        else:
            with tc.For_i(0, repeat, 1):
                body()
    nc.compile()
    _NC_CACHE[key] = nc
    return nc


def _emit(nc, tc, t, phases=3):
    with tc.tile_pool(name="xt", bufs=1) as xt_pool, \
         tc.tile_pool(name="xq", bufs=1) as xq_pool, \
         tc.tile_pool(name="wq", bufs=1) as wq_pool, \
         tc.tile_pool(name="wv", bufs=1) as wv_pool, \
         tc.tile_pool(name="gp", bufs=1) as g_pool, \
         tc.tile_pool(name="vp", bufs=1) as v_pool, \
         tc.tile_pool(name="small", bufs=1) as small:

        xth = [xt_pool.tile([128, 2, T], E4, tag=f"xth{d}", name=f"xth{d}")
               for d in range(DP)]
        xtl = [xt_pool.tile([128, 2, T], E4, tag=f"xtl{d}", name=f"xtl{d}")
               for d in range(DP)]
        xqh = [xq_pool.tile([128, 2, 1024], E4, tag=f"xqh{d}", name=f"xqh{d}")
               for d in range(DP)]
        xql = [xq_pool.tile([128, 2, 1024], E4, tag=f"xql{d}", name=f"xql{d}")
               for d in range(DP)]
        wqh = [wq_pool.tile([128, 2, 1024], E4, tag=f"wqh{d}", name=f"wqh{d}")
               for d in range(DP)]
        wql = [wq_pool.tile([128, 2, 1024], E4, tag=f"wql{d}", name=f"wql{d}")
               for d in range(DP)]
        wvh = [wv_pool.tile([128, 2, 1024], E4, tag=f"wvh{d}", name=f"wvh{d}")
               for d in range(DP)]
        wvl = [wv_pool.tile([128, 2, 1024], E4, tag=f"wvl{d}", name=f"wvl{d}")
               for d in range(DP)]
        qph = [g_pool.tile([128, 2, 1024], E4, tag=f"qph{m}", name=f"qph{m}")
               for m in range(DP)]
        qpl = [g_pool.tile([128, 2, 1024], E4, tag=f"qpl{m}", name=f"qpl{m}")
               for m in range(DP)]
        vph = [v_pool.tile([128, 2, 1024], E4, tag=f"vph{p}", name=f"vph{p}")
               for p in range(NT // 2)]
        vpl = [v_pool.tile([128, 2, 1024], E4, tag=f"vpl{p}", name=f"vpl{p}")
               for p in range(NT // 2)]
        tst = small.tile([2, 2, 1024], E4, tag="tst", name="tst_sb")
        mov = small.tile([2, NBLK, 2, 512], E4, tag="mov", name="mov_sb")
        dmsk = small.tile([128, 2, 2, 128], F32, tag="dmsk", name="dmsk_sb")
        ones64 = small.tile([128, 2, 1], E4, tag="ones", name="ones64")
        cbias = small.tile([128, 1], F32, tag="cb", name="cbias")
        nc.vector.memset(ones64[:], SV)
        nc.vector.memset(cbias[:], -CSHIFT)
        wrm = small.tile([128, 1], F32, tag="wrm", name="wrm")
        nc.scalar.activation(wrm[:], cbias[:],
                             mybir.ActivationFunctionType.Exp)

        # ---- DMA choreography (SP queue is in-order) ----
        # Q cold start first, then the rest in consumption order.
        nc.sync.dma_start(wqh[0][:, :, 0:384], t["wqh"][0, :, :, 0:384])
        nc.sync.dma_start(xqh[0][:, :, 0:512], t["xqh"][0, :, :, 0:512])
        nc.sync.dma_start(xqh[0][:, :, 512:1024], t["xqh"][0, :, :, 512:1024])
        nc.sync.dma_start(wqh[0][:, :, 384:1024], t["wqh"][0, :, :, 384:1024])
        nc.sync.dma_start(wql[0][:], t["wql"][0, :, :, :])
        nc.sync.dma_start(xql[0][:], t["xql"][0, :, :, :])
        for d in range(1, DP):
            nc.sync.dma_start(wqh[d][:], t["wqh"][d, :, :, :])
            nc.sync.dma_start(xqh[d][:], t["xqh"][d, :, :, :])
            nc.sync.dma_start(wql[d][:], t["wql"][d, :, :, :])
            nc.sync.dma_start(xql[d][:], t["xql"][d, :, :, :])
        for d in range(DP):
            nc.sync.dma_start(wvh[d][:], t["wvh"][d, :, :, :])
        for d in range(DP):
            nc.sync.dma_start(xth[d][:, :, 0:1024], t["xth"][d, :, :, 0:1024])
        for d in range(DP):
            nc.sync.dma_start(wvl[d][:], t["wvl"][d, :, :, :])
        for d in range(DP):
            nc.sync.dma_start(xtl[d][:, :, 0:1024], t["xtl"][d, :, :, 0:1024])
        for d in range(DP):
            nc.sync.dma_start(xth[d][:, :, 1024:2048], t["xth"][d, :, :, 1024:2048])
            nc.sync.dma_start(xtl[d][:, :, 1024:2048], t["xtl"][d, :, :, 1024:2048])
        nc.sync.dma_start(tst[:], t["tst"][:, :, :])
        nc.sync.dma_start(mov[:], t["mov_sel"][:, :, :, :])
        # dmask dram [2,2,128,128] -> sbuf [128, 2, 2, 128]
        nc.sync.dma_start(dmsk[:], t["dmask"].rearrange("a b p c -> p a b c"))

        pt_cm = tc.tile_pool(name="ptp", bufs=1)
        trans_cm = tc.tile_pool(name="trans", bufs=1)
        psS_cm = tc.tile_pool(name="psS", bufs=1, space="PSUM")
        pt_pool = pt_cm.__enter__()
        trans = trans_cm.__enter__()
        psS_pool = psS_cm.__enter__()
        psA_cm = tc.tile_pool(name="psA", bufs=1, space="PSUM")
        psA = psA_cm.__enter__()

        # ---- Phase Q: G = x_q @ Wqk_s, stored as e4m3 hi+lo pairs ----
        CHAINS_Q = ((wqh, xqh), (wql, xqh), (wqh, xql))

        def q_copies(m, half, ps):
            mp, sub = m // 2, m % 2
            dst_h = qph[mp][:, sub, ds(512 * half, 512)]
            dst_l = qpl[mp][:, sub, ds(512 * half, 512)]
            nc.scalar.activation(dst_h, ps[:],
                                 mybir.ActivationFunctionType.Copy)
            nc.vector.tensor_sub(dst_l, ps[:], dst_h)

        # group0: dp-outer to stream arriving tiles; last dp sweep m-first
        grp = ((0, 0), (0, 1), (1, 0), (1, 1), (2, 0), (2, 1))
        pss = {mh: psA.tile([128, 512], F32, tag="A", name="psQ_t", bufs=6)
               for mh in grp}
        for dp in range(DP - 1):
            for ci, (lh, rh) in enumerate(CHAINS_Q):
                for (m, half) in grp:
                    nc.tensor.matmul(
                        pss[(m, half)][:],
                        lh[dp][:, :, ts(m, 128)],
                        rh[dp][:, :, ds(512 * half, 512)],
                        start=(dp == 0 and ci == 0),
                        stop=False,
                        perf_mode=DR)
        for (m, half) in grp:
            for ci, (lh, rh) in enumerate(CHAINS_Q):
                nc.tensor.matmul(
                    pss[(m, half)][:],
                    lh[DP - 1][:, :, ts(m, 128)],
                    rh[DP - 1][:, :, ds(512 * half, 512)],
                    start=False, stop=(ci == 2),
                    perf_mode=DR)
            q_copies(m, half, pss[(m, half)])
        # m 3..7: streamed half-tiles (all inputs resident by now)
        for m in range(3, 8):
            for half in range(2):
                ps = psA.tile([128, 512], F32, tag="A", name="psQ_t", bufs=6)
                for dp in range(DP):
                    for ci, (lh, rh) in enumerate(CHAINS_Q):
                        nc.tensor.matmul(
                            ps[:],
                            lh[dp][:, :, ts(m, 128)],
                            rh[dp][:, :, ds(512 * half, 512)],
                            start=(dp == 0 and ci == 0),
                            stop=(dp == DP - 1 and ci == 2),
                            perf_mode=DR)
                q_copies(m, half, ps)

        # ---- Phase V: VP = x @ Wvp_s (full batch), e4m3 hi+lo pairs ----
        CHAINS_V = ((xth, wvh), (xtl, wvh), (xth, wvl))
        for tt in range(NT):
            tp, sub = tt // 2, tt % 2
            for half in range(2):
                psV = psA.tile([128, 512], F32, tag="A", name="psV_t", bufs=6)
                for ci, (lh, rh) in enumerate(CHAINS_V):
                    for dp in range(DP):
                        nc.tensor.matmul(
                            psV[:],
                            lh[dp][:, :, ts(tt, 128)],
                            rh[dp][:, :, ds(512 * half, 512)],
                            start=(ci == 0 and dp == 0),
                            stop=(ci == 2 and dp == DP - 1),
                            perf_mode=DR)
                dst_h = vph[tp][:, sub, ds(512 * half, 512)]
                dst_l = vpl[tp][:, sub, ds(512 * half, 512)]
                nc.scalar.activation(dst_h, psV[:],
                                     mybir.ActivationFunctionType.Copy)
                nc.vector.tensor_sub(dst_l, psV[:], dst_h)

        if phases <= 1:
            psA_cm.__exit__(None, None, None)
            with tc.tile_pool(name="dump", bufs=1) as dump:
                tk = dump.tile([128, 512], F16, tag="tk", name="tk")
                nc.vector.tensor_copy(tk[:], vph[0][:, 0, 0:512])
                nc.sync.dma_start(t["out"][0:128, 0:512], tk[:])
            return

        # ---- Phase B: scores -> exp split -> PV + den, software-pipelined ----
        # The first pair's scores are emitted while the psA pool is still
        # open (psS takes the 2 spare PSUM banks), so phase B overlaps the V
        # tail; po/pden pools open only after psA closes.
        blk_base = {}
        acc = 0
        for P in PAIR_ORDER:
            blk_base[P] = acc
            acc += CP[P] // 2

        po_pool = pden_pool = None

        if True:
            CHAINS_S = ((xth, qph), (xtl, qph), (xth, qpl))

            def emit_scores(P):
                cp = CP[P]
                blocks = cp // 2
                pts = []
                for tjp in range(blocks):
                    bid = blk_base[P] + tjp
                    psS = psS_pool.tile([128, 2, 256], F32, tag="s",
                                        name="psS_t", bufs=2)
                    nc.tensor.matmul(psS[:, :, :], tst[:, :, ts(tjp, 128)],
                                     mov[:, bid, :, :], start=True, stop=False,
                                     perf_mode=DR)
                    wS = 128 if tjp == blocks - 1 else 256
                    for i in range(2):
                        tj = 2 * tjp + i
                        for dp in range(DP):
                            for ci, (lh, rh) in enumerate(CHAINS_S):
                                nc.tensor.matmul(
                                    psS[:, i, 0:wS],
                                    lh[dp][:, :, ts(tj, 128)],
                                    rh[dp][:, :, ds(P * 256, wS)],
                                    start=False,
                                    stop=(dp == DP - 1 and ci == 2),
                                    perf_mode=DR)
                    p32 = trans.tile([128, 2, 256], F32, tag="p32",
                                     name="p32_t", bufs=4)
                    nc.scalar.activation(p32[:, :, :], psS[:, :, :],
                                         mybir.ActivationFunctionType.Exp,
                                         bias=cbias[:, 0:1], scale=1.0 / SQ)
                    if tjp == blocks - 2:      # block A masks
                        nc.vector.tensor_mul(p32[:, 0, 128:256],
                                             p32[:, 0, 128:256],
                                             dmsk[:, 0, 0, :])
                        nc.vector.tensor_mul(p32[:, 1, 128:256],
                                             p32[:, 1, 128:256],
                                             dmsk[:, 0, 1, :])
                    if tjp == blocks - 1:      # block B masks
                        nc.vector.tensor_mul(p32[:, 0, 0:128],
                                             p32[:, 0, 0:128],
                                             dmsk[:, 1, 0, :])
                        nc.vector.tensor_mul(p32[:, 1, 0:128],
                                             p32[:, 1, 0:128],
                                             dmsk[:, 1, 1, :])
                    pth = pt_pool.tile([128, 2, 256], E5, tag=f"pth{tjp}",
                                       name="pth_t", bufs=2)
                    ptl = pt_pool.tile([128, 2, 256], E5, tag=f"ptl{tjp}",
                                       name="ptl_t", bufs=2)
                    nc.scalar.activation(pth[:, :, :], p32[:, :, :],
                                         mybir.ActivationFunctionType.Copy)
                    nc.vector.tensor_sub(ptl[:, :, :], p32[:, :, :],
                                         pth[:, :, :])
                    pts.append((pth, ptl))
                return pts

            def emit_pv(P, pts):
                if phases <= 2:
                    return
                cp = CP[P]
                blocks = cp // 2
                for slot in range(2):
                    nblk = blocks if slot == 0 else blocks - 1
                    row = 128 * (2 * P + slot)
                    pden = pden_pool.tile([128, 1], F32, tag=f"pd{slot}",
                                          name=f"pden{slot}_t", bufs=1)
                    rec = trans.tile([128, 1], F32, tag="rec", name="rec_t",
                                     bufs=4)
                    ob = trans.tile([128, 1024], F16, tag="ob",
                                    name="ob_t", bufs=3)
                    for half in range(2):
                        po = po_pool.tile([128, 512], F32, tag="po",
                                          name="po_t", bufs=2)
                        for bi in range(nblk):
                            ph, pl = pts[bi]
                            sh = ph[:, :, ds(slot * 128, 128)]
                            sl = pl[:, :, ds(slot * 128, 128)]
                            if half == 0:
                                nc.tensor.matmul(pden[:], sh, ones64[:, :, :],
                                                 start=(bi == 0), stop=False,
                                                 perf_mode=DR)
                                nc.tensor.matmul(pden[:], sl, ones64[:, :, :],
                                                 start=False,
                                                 stop=(bi == nblk - 1),
                                                 perf_mode=DR)
                            mh = vph[bi][:, :, ds(512 * half, 512)]
                            ml = vpl[bi][:, :, ds(512 * half, 512)]
                            for ci, (s_, m_) in enumerate(
                                    ((sh, mh), (sl, mh), (sh, ml))):
                                nc.tensor.matmul(
                                    po[:], s_, m_,
                                    start=(bi == 0 and ci == 0),
                                    stop=(bi == nblk - 1 and ci == 2),
                                    perf_mode=DR)
                        if half == 0:
                            nc.vector.reciprocal(rec[:], pden[:])
                            nc.scalar.activation(
                                ob[:, 0:512], po[:],
                                mybir.ActivationFunctionType.Copy,
                                bias=0.0, scale=rec[:, 0:1])
                        else:
                            nc.vector.tensor_scalar_mul(ob[:, 512:1024], po[:],
                                                        rec[:, 0:1])
                    nc.sync.dma_start(t["out"][ds(row, 128), :], ob[:])

            prev = None
            for P in PAIR_ORDER:
                pts = emit_scores(P)
                if po_pool is None:
                    # first pair's scores emitted; now V is long done --
                    # release psA and claim its banks for PV accumulators
                    psA_cm.__exit__(None, None, None)
                    po_cm = tc.tile_pool(name="poP", bufs=1, space="PSUM")
                    pden_cm = tc.tile_pool(name="pdn", bufs=1, space="PSUM")
                    po_pool = po_cm.__enter__()
                    pden_pool = pden_cm.__enter__()
                if prev is not None:
                    emit_pv(*prev)
                prev = (P, pts)
            emit_pv(*prev)
            pden_cm.__exit__(None, None, None)
            po_cm.__exit__(None, None, None)
        psS_cm.__exit__(None, None, None)
        trans_cm.__exit__(None, None, None)
        pt_cm.__exit__(None, None, None)


def _split8(a):
    """f32 array -> (hi, lo) e4m3 pair with unscaled residual."""
    hi = a.astype(NE4)
    lo = (a - hi.astype(np.float32)).astype(NE4)
    return hi, lo


def _wlayout(w):
    # [1024 (d), 1024 (m)] -> [DP, 128, 2, 1024]: pair d-chunks for DoubleRow
    return np.ascontiguousarray(
        w.reshape(DP, 2, 128, 1024).transpose(0, 2, 1, 3))


def _host_consts(par):
    """mov_sel (t-init moving patterns) and dmask for a parity."""
    v = np.zeros((4, 4, 512), np.float32)
    for k in range(4):
        v[k, 0, 0:256] = 1.0
        v[k, 1, 256:512] = 1.0
    v[1, 2, 384:512] = 240.0                      # parity0 block A
    v[2, 2, 128:256] = 240.0                      # parity0 block B
    v[2, 2, 384:512] = 240.0
    v[3, 2, 128:512] = 240.0                      # parity1 block B
    mov = np.zeros((NBLK, 4, 512), np.float32)
    bi = 0
    for P in PAIR_ORDER:
        blocks = CP[P] // 2
        for tjp in range(blocks):
            if par == 0 and tjp == blocks - 2:
                sel = 1
            elif par == 0 and tjp == blocks - 1:
                sel = 2
            elif par == 1 and tjp == blocks - 1:
                sel = 3
            else:
                sel = 0
            mov[bi] = v[sel]
            bi += 1
    assert bi == NBLK
    # DoubleRow layout [2(k), NBLK, 2(i), 512]: k0i0=row0, k1i0=row2(kill),
    # k0i1=row1, k1i1=0
    mov_dr = np.zeros((2, NBLK, 2, 512), np.float32)
    mov_dr[0, :, 0, :] = mov[:, 0, :]
    mov_dr[1, :, 0, :] = mov[:, 2, :]
    mov_dr[0, :, 1, :] = mov[:, 1, :]
    mov = mov_dr

    tri = (np.arange(128)[:, None] <= np.arange(128)[None, :]).astype(np.float32)
    ones = np.ones((128, 128), np.float32)
    dm = np.empty((2, 2, 128, 128), np.float32)
    if par == 0:
        dm[0, 0], dm[0, 1] = tri, ones            # block A: diag at [i=0,slot1]
        dm[1, 0], dm[1, 1] = ones, tri            # block B: diag at [i=1,slot0]
    else:
        dm[0, 0], dm[0, 1] = ones, tri            # block A: diag at [i=1,slot1]
        dm[1, 0], dm[1, 1] = tri, ones            # block B: diag at [i=0,slot0]
    return mov.astype(NE4), dm


def kernel(x, W_attn, b_attn, W_proj, b_proj, _repeat=1, _results_only=False,
           _phases=3):
    x = np.asarray(x, np.float32)
    W_attn = np.asarray(W_attn, np.float64)
    b_attn = np.asarray(b_attn, np.float64)
    W_proj = np.asarray(W_proj, np.float64)
    b_proj = np.asarray(b_proj, np.float64)
    B = x.shape[0]

    nc = _build(_repeat, _phases)

    Wq = W_attn[:, :D]
    Wk = W_attn[:, D:2 * D]
    Wv = W_attn[:, 2 * D:]
    wqk_s = np.float32((Wq * 0.125) @ Wk.T * SQ)
    wvp_s = np.float32(Wv @ W_proj * SV)
    bqk = (b_attn[:D] * 0.125) @ Wk.T            # [D], f64
    b_eff = np.float32(b_proj + b_attn[2 * D:] @ W_proj)

    wqh, wql = _split8(wqk_s)
    wvh, wvl = _split8(wvp_s)
    wqh, wql = _wlayout(wqh), _wlayout(wql)
    wvh, wvl = _wlayout(wvh), _wlayout(wvl)
    consts = [_host_consts(0), _host_consts(1)]

    def xlayout(a, n):
        # [1024 (d), n (tok)] e4 -> [DP, 128, 2, n]
        return np.ascontiguousarray(
            a.reshape(DP, 2, 128, n).transpose(0, 2, 1, 3))

    in_maps = []
    for c in range(8):
        b, par = c // 2, c % 2
        own = OWN[par]
        xTb = np.ascontiguousarray(x[b].T)       # [D, T] f32
        xh, xl = _split8(xTb)
        cols = np.concatenate([np.arange(128 * q, 128 * (q + 1)) for q in own])
        tvec = np.float32(x[b].astype(np.float64) @ bqk) * np.float32(SQ)
        trows = tvec.reshape(8, 2, 128).transpose(1, 0, 2).reshape(2, 1024)
        tst = np.zeros((2, 2, 1024), np.float32)
        tst[0, 0] = trows[0]        # k0,i0: t(tj0)
        tst[1, 0] = -240.0          # k1,i0: kill row
        tst[0, 1] = trows[1]        # k0,i1: t(tj1)
        mov, dm = consts[par]
        in_maps.append({
            "xth": xlayout(xh, T), "xtl": xlayout(xl, T),
            "xqh": xlayout(np.ascontiguousarray(xh[:, cols]), 1024),
            "xql": xlayout(np.ascontiguousarray(xl[:, cols]), 1024),
            "wqh": wqh, "wql": wql, "wvh": wvh, "wvl": wvl,
            "tst": tst.astype(NE4), "mov_sel": mov, "dmask": dm,
        })

    res = run_bass_kernel_spmd(nc, in_maps, core_ids=list(range(8)))
    if _results_only:
        return res

    out = np.empty((B, T, D), np.float32)
    for c in range(8):
        b, par = c // 2, c % 2
        part = res.results[c]["out"].astype(np.float32)
        for s, q in enumerate(OWN[par]):
            out[b, 128 * q:128 * (q + 1), :] = part[128 * s:128 * (s + 1), :] + b_eff
    return out


# revision 5
# speedup vs baseline: 1.0104x; 1.0062x over previous
"""Trainium2 Bass kernel for nn_CausalSelfAttention_8237747274097 — v2.

All-fp8 DoubleRow rewrite with residual compensation.

Math (exact folds, as v1):
    qkv = x @ W_attn + b_attn ; q,k,v = split ; single-head attention.
    Wqk = (Wq/8) @ Wk^T folded (no K projection);  Wvp = Wv @ W_proj folded
    (no output projection); k-bias drops, v-bias folds into host b_eff.
    Per-key score bias t_j = (bq/8)·Wk^T·x_j enters via a K=4 init matmul.

Numerics: every matmul runs as fp8e4m3 DoubleRow (2 k-chunks per
instruction, 0.5 cycles/out-elem) with hi+lo residual compensation:
for operands A≈Ah+Al, B≈Bh+Bl the product uses 3 chains Ah·Bh + Al·Bh +
Ah·Bl accumulated in one PSUM (residuals are UNSCALED e4m3 — fp8
subnormals verified exact on HW).  exp outputs are split as e5m2 hi+lo
(ph + pl = p exactly to ~1.6%).  Verified end-to-end metric 2.8e-3 vs
the 2e-2 gate in numpy emulation.

Scales (exact pow-2): Wqk*512 (scores descaled inside exp), Wvp*64
(descaled via the den=64*sum(p) reciprocal).  Global exp shift -2.75
cancels in softmax and keeps exp(s) inside e5m2 range.

Causal masking: dead 128-col regions are killed by the t-init matmul
itself (row2 = -240 times a data-selected 240-pattern => psS <= -5e4 =>
exp == 0 exactly); diagonal tiles get a triangular multiply on the f32
exp output.  All parity differences are DATA (mov_sel / dmask), so one
NEFF serves all 8 cores.

Sharding (unchanged from v1): core c = (batch c//2, parity c%2); each
core owns 8 of 16 query row-tiles (OWN), computes full V for its batch.
"""

import numpy as np
import ml_dtypes

import concourse.bass as bass
import concourse.tile as tile
import concourse.mybir as mybir
from concourse import bacc
from concourse.bass import ts, ds
from concourse.bass_utils import run_bass_kernel_spmd

F32 = mybir.dt.float32
F16 = mybir.dt.float16
E4 = mybir.dt.float8e4
E5 = mybir.dt.float8e5
DR = mybir.MatmulPerfMode.DoubleRow
NE4 = ml_dtypes.float8_e4m3
NE5 = ml_dtypes.float8_e5m2

T, D = 2048, 1024
NT = T // 128          # 16 key/query tiles
DP = 4                 # d-chunk pairs (8 chunks of 128, DoubleRow-paired)
OWN = [[15, 12, 11, 8, 7, 4, 3, 0],
       [14, 13, 10, 9, 6, 5, 2, 1]]
CP = [16, 12, 8, 4]    # j-blocks per pair P (uniform across cores)
PAIR_ORDER = (0, 1, 2, 3)
SQ = 512.0
SV = 64.0
CSHIFT = 2.75          # exact in fp8/f32; exp(s - CSHIFT)
NBLK = sum(CP) // 2    # 20 tj-pair blocks per core

_NC_CACHE = {}


def _build(repeat=1, phases=3):
    key = (repeat, phases)
    if key in _NC_CACHE:
        return _NC_CACHE[key]
    nc = bacc.Bacc("TRN2", target_bir_lowering=False, debug=False,
                   enable_asserts=False, num_devices=8)
    t = {}
    for nm in ("xth", "xtl"):
        t[nm] = nc.dram_tensor(nm, [DP, 128, 2, T], E4, kind="ExternalInput").ap()
    for nm in ("xqh", "xql", "wqh", "wql", "wvh", "wvl"):
        t[nm] = nc.dram_tensor(nm, [DP, 128, 2, 1024], E4, kind="ExternalInput").ap()
    t["tst"] = nc.dram_tensor("tst", [2, 2, 1024], E4, kind="ExternalInput").ap()
    t["mov_sel"] = nc.dram_tensor("mov_sel", [2, NBLK, 2, 512], E4,
                                  kind="ExternalInput").ap()
    t["dmask"] = nc.dram_tensor("dmask", [2, 2, 128, 128], F32,
                                kind="ExternalInput").ap()
    t["out"] = nc.dram_tensor("out", [1024, 1024], F16, kind="ExternalOutput").ap()

    with tile.TileContext(nc, pool_alloc_mode="queue") as tc:
        def body(_i=None):
            _emit(nc, tc, t, phases)
        if repeat == 1:
            body()
        else:
            with tc.For_i(0, repeat, 1):
                body()
    nc.compile()
    _NC_CACHE[key] = nc
    return nc


def _emit(nc, tc, t, phases=3):
    with tc.tile_pool(name="xt", bufs=1) as xt_pool, \
         tc.tile_pool(name="xq", bufs=1) as xq_pool, \
         tc.tile_pool(name="wq", bufs=1) as wq_pool, \
         tc.tile_pool(name="wv", bufs=1) as wv_pool, \
         tc.tile_pool(name="gp", bufs=1) as g_pool, \
         tc.tile_pool(name="vp", bufs=1) as v_pool, \
         tc.tile_pool(name="small", bufs=1) as small:

        xth = [xt_pool.tile([128, 2, T], E4, tag=f"xth{d}", name=f"xth{d}")
               for d in range(DP)]
        xtl = [xt_pool.tile([128, 2, T], E4, tag=f"xtl{d}", name=f"xtl{d}")
               for d in range(DP)]
        xqh = [xq_pool.tile([128, 2, 1024], E4, tag=f"xqh{d}", name=f"xqh{d}")
               for d in range(DP)]
        xql = [xq_pool.tile([128, 2, 1024], E4, tag=f"xql{d}", name=f"xql{d}")
               for d in range(DP)]
        wqh = [wq_pool.tile([128, 2, 1024], E4, tag=f"wqh{d}", name=f"wqh{d}")
               for d in range(DP)]
        wql = [wq_pool.tile([128, 2, 1024], E4, tag=f"wql{d}", name=f"wql{d}")
               for d in range(DP)]
        wvh = [wv_pool.tile([128, 2, 1024], E4, tag=f"wvh{d}", name=f"wvh{d}")
               for d in range(DP)]
        wvl = [wv_pool.tile([128, 2, 1024], E4, tag=f"wvl{d}", name=f"wvl{d}")
               for d in range(DP)]
        qph = [g_pool.tile([128, 2, 1024], E4, tag=f"qph{m}", name=f"qph{m}")
               for m in range(DP)]
        qpl = [g_pool.tile([128, 2, 1024], E4, tag=f"qpl{m}", name=f"qpl{m}")
               for m in range(DP)]
        vph = [v_pool.tile([128, 2, 1024], E4, tag=f"vph{p}", name=f"vph{p}")
               for p in range(NT // 2)]
        vpl = [v_pool.tile([128, 2, 1024], E4, tag=f"vpl{p}", name=f"vpl{p}")
               for p in range(NT // 2)]
        tst = small.tile([2, 2, 1024], E4, tag="tst", name="tst_sb")
        mov = small.tile([2, NBLK, 2, 512], E4, tag="mov", name="mov_sb")
        dmsk = small.tile([128, 2, 2, 128], F32, tag="dmsk", name="dmsk_sb")
        ones64 = small.tile([128, 2, 1], E4, tag="ones", name="ones64")
        cbias = small.tile([128, 1], F32, tag="cb", name="cbias")
        nc.vector.memset(ones64[:], SV)
        nc.vector.memset(cbias[:], -CSHIFT)
        wrm = small.tile([128, 1], F32, tag="wrm", name="wrm")
        nc.scalar.activation(wrm[:], cbias[:],
                             mybir.ActivationFunctionType.Exp)

        # ---- DMA choreography (SP queue is in-order) ----
        # Q cold start first, then the rest in consumption order.
        nc.sync.dma_start(wqh[0][:, :, 0:384], t["wqh"][0, :, :, 0:384])
        nc.sync.dma_start(xqh[0][:, :, 0:512], t["xqh"][0, :, :, 0:512])
        nc.sync.dma_start(xqh[0][:, :, 512:1024], t["xqh"][0, :, :, 512:1024])
        nc.sync.dma_start(wqh[0][:, :, 384:1024], t["wqh"][0, :, :, 384:1024])
        nc.sync.dma_start(wql[0][:], t["wql"][0, :, :, :])
        nc.sync.dma_start(xql[0][:], t["xql"][0, :, :, :])
        for d in range(1, DP):
            nc.sync.dma_start(wqh[d][:], t["wqh"][d, :, :, :])
            nc.sync.dma_start(xqh[d][:], t["xqh"][d, :, :, :])
            nc.sync.dma_start(wql[d][:], t["wql"][d, :, :, :])
            nc.sync.dma_start(xql[d][:], t["xql"][d, :, :, :])
        for d in range(DP):
            nc.sync.dma_start(wvh[d][:], t["wvh"][d, :, :, :])
        for d in range(DP):
            nc.sync.dma_start(xth[d][:, :, 0:1024], t["xth"][d, :, :, 0:1024])
        for d in range(DP):
            nc.sync.dma_start(wvl[d][:], t["wvl"][d, :, :, :])
        for d in range(DP):
            nc.sync.dma_start(xtl[d][:, :, 0:1024], t["xtl"][d, :, :, 0:1024])
        for d in range(DP):
            nc.sync.dma_start(xth[d][:, :, 1024:2048], t["xth"][d, :, :, 1024:2048])
            nc.sync.dma_start(xtl[d][:, :, 1024:2048], t["xtl"][d, :, :, 1024:2048])
        nc.sync.dma_start(tst[:], t["tst"][:, :, :])
        nc.sync.dma_start(mov[:], t["mov_sel"][:, :, :, :])
        # dmask dram [2,2,128,128] -> sbuf [128, 2, 2, 128]
        nc.sync.dma_start(dmsk[:], t["dmask"].rearrange("a b p c -> p a b c"))

        pt_cm = tc.tile_pool(name="ptp", bufs=1)
        trans_cm = tc.tile_pool(name="trans", bufs=1)
        psS_cm = tc.tile_pool(name="psS", bufs=1, space="PSUM")
        pt_pool = pt_cm.__enter__()
        trans = trans_cm.__enter__()
        psS_pool = psS_cm.__enter__()
        psA_cm = tc.tile_pool(name="psA", bufs=1, space="PSUM")
        psA = psA_cm.__enter__()

        # ---- Phase Q: G = x_q @ Wqk_s, stored as e4m3 hi+lo pairs ----
        CHAINS_Q = ((wqh, xqh), (wql, xqh), (wqh, xql))

        def q_copies(m, half, ps):
            mp, sub = m // 2, m % 2
            dst_h = qph[mp][:, sub, ds(512 * half, 512)]
            dst_l = qpl[mp][:, sub, ds(512 * half, 512)]
            nc.scalar.activation(dst_h, ps[:],
                                 mybir.ActivationFunctionType.Copy)
            nc.vector.tensor_sub(dst_l, ps[:], dst_h)

        # group0: dp-outer to stream arriving tiles; last dp sweep m-first
        grp = ((0, 0), (0, 1), (1, 0), (1, 1), (2, 0), (2, 1))
        pss = {mh: psA.tile([128, 512], F32, tag="A", name="psQ_t", bufs=6)
               for mh in grp}
        for dp in range(DP - 1):
            for ci, (lh, rh) in enumerate(CHAINS_Q):
                for (m, half) in grp:
                    nc.tensor.matmul(
                        pss[(m, half)][:],
                        lh[dp][:, :, ts(m, 128)],
                        rh[dp][:, :, ds(512 * half, 512)],
                        start=(dp == 0 and ci == 0),
                        stop=False,
                        perf_mode=DR)
        for (m, half) in grp:
            for ci, (lh, rh) in enumerate(CHAINS_Q):
                nc.tensor.matmul(
                    pss[(m, half)][:],
                    lh[DP - 1][:, :, ts(m, 128)],
                    rh[DP - 1][:, :, ds(512 * half, 512)],
                    start=False, stop=(ci == 2),
                    perf_mode=DR)
            q_copies(m, half, pss[(m, half)])
        # m 3..7: streamed half-tiles (all inputs resident by now)
        for m in range(3, 8):
            for half in range(2):
                ps = psA.tile([128, 512], F32, tag="A", name="psQ_t", bufs=6)
                for dp in range(DP):
                    for ci, (lh, rh) in enumerate(CHAINS_Q):
                        nc.tensor.matmul(
                            ps[:],
                            lh[dp][:, :, ts(m, 128)],
                            rh[dp][:, :, ds(512 * half, 512)],
                            start=(dp == 0 and ci == 0),
                            stop=(dp == DP - 1 and ci == 2),
                            perf_mode=DR)
                q_copies(m, half, ps)

        # ---- Phase V: VP = x @ Wvp_s (full batch), e4m3 hi+lo pairs ----
        CHAINS_V = ((xth, wvh), (xtl, wvh), (xth, wvl))
        for tt in range(NT):
            tp, sub = tt // 2, tt % 2
            for half in range(2):
                psV = psA.tile([128, 512], F32, tag="A", name="psV_t", bufs=6)
                for ci, (lh, rh) in enumerate(CHAINS_V):
                    for dp in range(DP):
                        nc.tensor.matmul(
                            psV[:],
                            lh[dp][:, :, ts(tt, 128)],
                            rh[dp][:, :, ds(512 * half, 512)],
                            start=(ci == 0 and dp == 0),
                            stop=(ci == 2 and dp == DP - 1),
                            perf_mode=DR)
                dst_h = vph[tp][:, sub, ds(512 * half, 512)]
                dst_l = vpl[tp][:, sub, ds(512 * half, 512)]
                nc.scalar.activation(dst_h, psV[:],
                                     mybir.ActivationFunctionType.Copy)
                nc.vector.tensor_sub(dst_l, psV[:], dst_h)

        if phases <= 1:
            psA_cm.__exit__(None, None, None)
            with tc.tile_pool(name="dump", bufs=1) as dump:
                tk = dump.tile([128, 512], F16, tag="tk", name="tk")
                nc.vector.tensor_copy(tk[:], vph[0][:, 0, 0:512])
                nc.sync.dma_start(t["out"][0:128, 0:512], tk[:])
            return

        # ---- Phase B: scores -> exp split -> PV + den, software-pipelined ----
        # The first pair's scores are emitted while the psA pool is still
        # open (psS takes the 2 spare PSUM banks), so phase B overlaps the V
        # tail; po/pden pools open only after psA closes.
        blk_base = {}
        acc = 0
        for P in PAIR_ORDER:
            blk_base[P] = acc
            acc += CP[P] // 2

        po_pool = pden_pool = None

        if True:
            CHAINS_S = ((xth, qph), (xtl, qph), (xth, qpl))

            def emit_scores(P):
                cp = CP[P]
                blocks = cp // 2
                pts = []
                for tjp in range(blocks):
                    bid = blk_base[P] + tjp
                    psS = psS_pool.tile([128, 2, 256], F32, tag="s",
                                        name="psS_t", bufs=2)
                    nc.tensor.matmul(psS[:, :, :], tst[:, :, ts(tjp, 128)],
                                     mov[:, bid, :, :], start=True, stop=False,
                                     perf_mode=DR)
                    wS = 128 if tjp == blocks - 1 else 256
                    for i in range(2):
                        tj = 2 * tjp + i
                        for dp in range(DP):
                            for ci, (lh, rh) in enumerate(CHAINS_S):
                                nc.tensor.matmul(
                                    psS[:, i, 0:wS],
                                    lh[dp][:, :, ts(tj, 128)],
                                    rh[dp][:, :, ds(P * 256, wS)],
                                    start=False,
                                    stop=(dp == DP - 1 and ci == 2),
                                    perf_mode=DR)
                    p32 = trans.tile([128, 2, 256], F32, tag="p32",
                                     name="p32_t", bufs=4)
                    nc.scalar.activation(p32[:, :, :], psS[:, :, :],
                                         mybir.ActivationFunctionType.Exp,
                                         bias=cbias[:, 0:1], scale=1.0 / SQ)
                    if tjp == blocks - 2:      # block A masks
                        nc.vector.tensor_mul(p32[:, 0, 128:256],
                                             p32[:, 0, 128:256],
                                             dmsk[:, 0, 0, :])
                        nc.vector.tensor_mul(p32[:, 1, 128:256],
                                             p32[:, 1, 128:256],
                                             dmsk[:, 0, 1, :])
                    if tjp == blocks - 1:      # block B masks
                        nc.vector.tensor_mul(p32[:, 0, 0:128],
                                             p32[:, 0, 0:128],
                                             dmsk[:, 1, 0, :])
                        nc.vector.tensor_mul(p32[:, 1, 0:128],
                                             p32[:, 1, 0:128],
                                             dmsk[:, 1, 1, :])
                    pth = pt_pool.tile([128, 2, 256], E5, tag=f"pth{tjp}",
                                       name="pth_t", bufs=2)
                    ptl = pt_pool.tile([128, 2, 256], E5, tag=f"ptl{tjp}",
                                       name="ptl_t", bufs=2)
                    nc.scalar.activation(pth[:, :, :], p32[:, :, :],
                                         mybir.ActivationFunctionType.Copy)
                    nc.vector.tensor_sub(ptl[:, :, :], p32[:, :, :],
                                         pth[:, :, :])
                    pts.append((pth, ptl))
                return pts

            def emit_pv(P, pts):
                if phases <= 2:
                    return
                cp = CP[P]
                blocks = cp // 2
                for slot in range(2):
                    nblk = blocks if slot == 0 else blocks - 1
                    row = 128 * (2 * P + slot)
                    pden = pden_pool.tile([128, 1], F32, tag=f"pd{slot}",
                                          name=f"pden{slot}_t", bufs=1)
                    rec = trans.tile([128, 1], F32, tag="rec", name="rec_t",
                                     bufs=4)
                    ob = trans.tile([128, 1024], F16, tag="ob",
                                    name="ob_t", bufs=3)
                    for half in range(2):
                        po = po_pool.tile([128, 512], F32, tag="po",
                                          name="po_t", bufs=2)
                        for bi in range(nblk):
                            ph, pl = pts[bi]
                            sh = ph[:, :, ds(slot * 128, 128)]
                            sl = pl[:, :, ds(slot * 128, 128)]
                            if half == 0:
                                nc.tensor.matmul(pden[:], sh, ones64[:, :, :],
                                                 start=(bi == 0), stop=False,
                                                 perf_mode=DR)
                                nc.tensor.matmul(pden[:], sl, ones64[:, :, :],
                                                 start=False,
                                                 stop=(bi == nblk - 1),
                                                 perf_mode=DR)
                            mh = vph[bi][:, :, ds(512 * half, 512)]
                            ml = vpl[bi][:, :, ds(512 * half, 512)]
                            for ci, (s_, m_) in enumerate(
                                    ((sh, mh), (sl, mh), (sh, ml))):
                                nc.tensor.matmul(
                                    po[:], s_, m_,
                                    start=(bi == 0 and ci == 0),
                                    stop=(bi == nblk - 1 and ci == 2),
                                    perf_mode=DR)
                        if half == 0:
                            nc.vector.reciprocal(rec[:], pden[:])
                            nc.scalar.activation(
                                ob[:, 0:512], po[:],
                                mybir.ActivationFunctionType.Copy,
                                bias=0.0, scale=rec[:, 0:1])
                        else:
                            nc.vector.tensor_scalar_mul(ob[:, 512:1024], po[:],
                                                        rec[:, 0:1])
                        if P == PAIR_ORDER[-1]:
                            # tail pair: per-half DMAs so half0 departs early
                            # and the final transfer on the critical path is
                            # half-size
                            nc.sync.dma_start(
                                t["out"][ds(row, 128), ds(512 * half, 512)],
                                ob[:, ds(512 * half, 512)])
                    if P != PAIR_ORDER[-1]:
                        nc.sync.dma_start(t["out"][ds(row, 128), :], ob[:])

            prev = None
            for P in PAIR_ORDER:
                pts = emit_scores(P)
                if po_pool is None:
                    # first pair's scores emitted; now V is long done --
                    # release psA and claim its banks for PV accumulators
                    psA_cm.__exit__(None, None, None)
                    po_cm = tc.tile_pool(name="poP", bufs=1, space="PSUM")
                    pden_cm = tc.tile_pool(name="pdn", bufs=1, space="PSUM")
                    po_pool = po_cm.__enter__()
                    pden_pool = pden_cm.__enter__()
                if prev is not None:
                    emit_pv(*prev)
                prev = (P, pts)
            emit_pv(*prev)
            pden_cm.__exit__(None, None, None)
            po_cm.__exit__(None, None, None)
        psS_cm.__exit__(None, None, None)
        trans_cm.__exit__(None, None, None)
        pt_cm.__exit__(None, None, None)


def _split8(a):
    """f32 array -> (hi, lo) e4m3 pair with unscaled residual."""
    hi = a.astype(NE4)
    lo = (a - hi.astype(np.float32)).astype(NE4)
    return hi, lo


def _wlayout(w):
    # [1024 (d), 1024 (m)] -> [DP, 128, 2, 1024]: pair d-chunks for DoubleRow
    return np.ascontiguousarray(
        w.reshape(DP, 2, 128, 1024).transpose(0, 2, 1, 3))


def _host_consts(par):
    """mov_sel (t-init moving patterns) and dmask for a parity."""
    v = np.zeros((4, 4, 512), np.float32)
    for k in range(4):
        v[k, 0, 0:256] = 1.0
        v[k, 1, 256:512] = 1.0
    v[1, 2, 384:512] = 240.0                      # parity0 block A
    v[2, 2, 128:256] = 240.0                      # parity0 block B
    v[2, 2, 384:512] = 240.0
    v[3, 2, 128:512] = 240.0                      # parity1 block B
    mov = np.zeros((NBLK, 4, 512), np.float32)
    bi = 0
    for P in PAIR_ORDER:
        blocks = CP[P] // 2
        for tjp in range(blocks):
            if par == 0 and tjp == blocks - 2:
                sel = 1
            elif par == 0 and tjp == blocks - 1:
                sel = 2
            elif par == 1 and tjp == blocks - 1:
                sel = 3
            else:
                sel = 0
            mov[bi] = v[sel]
            bi += 1
    assert bi == NBLK
    # DoubleRow layout [2(k), NBLK, 2(i), 512]: k0i0=row0, k1i0=row2(kill),
    # k0i1=row1, k1i1=0
    mov_dr = np.zeros((2, NBLK, 2, 512), np.float32)
    mov_dr[0, :, 0, :] = mov[:, 0, :]
    mov_dr[1, :, 0, :] = mov[:, 2, :]
    mov_dr[0, :, 1, :] = mov[:, 1, :]
    mov = mov_dr

    tri = (np.arange(128)[:, None] <= np.arange(128)[None, :]).astype(np.float32)
    ones = np.ones((128, 128), np.float32)
    dm = np.empty((2, 2, 128, 128), np.float32)
    if par == 0:
        dm[0, 0], dm[0, 1] = tri, ones            # block A: diag at [i=0,slot1]
        dm[1, 0], dm[1, 1] = ones, tri            # block B: diag at [i=1,slot0]
    else:
        dm[0, 0], dm[0, 1] = ones, tri            # block A: diag at [i=1,slot1]
        dm[1, 0], dm[1, 1] = tri, ones            # block B: diag at [i=0,slot0]
    return mov.astype(NE4), dm


def kernel(x, W_attn, b_attn, W_proj, b_proj, _repeat=1, _results_only=False,
           _phases=3):
    x = np.asarray(x, np.float32)
    W_attn = np.asarray(W_attn, np.float64)
    b_attn = np.asarray(b_attn, np.float64)
    W_proj = np.asarray(W_proj, np.float64)
    b_proj = np.asarray(b_proj, np.float64)
    B = x.shape[0]

    nc = _build(_repeat, _phases)

    Wq = W_attn[:, :D]
    Wk = W_attn[:, D:2 * D]
    Wv = W_attn[:, 2 * D:]
    wqk_s = np.float32((Wq * 0.125) @ Wk.T * SQ)
    wvp_s = np.float32(Wv @ W_proj * SV)
    bqk = (b_attn[:D] * 0.125) @ Wk.T            # [D], f64
    b_eff = np.float32(b_proj + b_attn[2 * D:] @ W_proj)

    wqh, wql = _split8(wqk_s)
    wvh, wvl = _split8(wvp_s)
    wqh, wql = _wlayout(wqh), _wlayout(wql)
    wvh, wvl = _wlayout(wvh), _wlayout(wvl)
    consts = [_host_consts(0), _host_consts(1)]

    def xlayout(a, n):
        # [1024 (d), n (tok)] e4 -> [DP, 128, 2, n]
        return np.ascontiguousarray(
            a.reshape(DP, 2, 128, n).transpose(0, 2, 1, 3))

    in_maps = []
    for c in range(8):
        b, par = c // 2, c % 2
        own = OWN[par]
        xTb = np.ascontiguousarray(x[b].T)       # [D, T] f32
        xh, xl = _split8(xTb)
        cols = np.concatenate([np.arange(128 * q, 128 * (q + 1)) for q in own])
        tvec = np.float32(x[b].astype(np.float64) @ bqk) * np.float32(SQ)
        trows = tvec.reshape(8, 2, 128).transpose(1, 0, 2).reshape(2, 1024)
        tst = np.zeros((2, 2, 1024), np.float32)
        tst[0, 0] = trows[0]        # k0,i0: t(tj0)
        tst[1, 0] = -240.0          # k1,i0: kill row
        tst[0, 1] = trows[1]        # k0,i1: t(tj1)
        mov, dm = consts[par]
        in_maps.append({
            "xth": xlayout(xh, T), "xtl": xlayout(xl, T),
            "xqh": xlayout(np.ascontiguousarray(xh[:, cols]), 1024),
            "xql": xlayout(np.ascontiguousarray(xl[:, cols]), 1024),
            "wqh": wqh, "wql": wql, "wvh": wvh, "wvl": wvl,
            "tst": tst.astype(NE4), "mov_sel": mov, "dmask": dm,
        })

    res = run_bass_kernel_spmd(nc, in_maps, core_ids=list(range(8)))
    if _results_only:
        return res

    out = np.empty((B, T, D), np.float32)
    for c in range(8):
        b, par = c // 2, c % 2
        part = res.results[c]["out"].astype(np.float32)
        for s, q in enumerate(OWN[par]):
            out[b, 128 * q:128 * (q + 1), :] = part[128 * s:128 * (s + 1), :] + b_eff
    return out
